# revision 1
# baseline (speedup 1.0000x reference)
"""DAG-GNN level-sweep kernel for Trainium2 (8 NeuronCores, Bass/Tile).

Structure:
  - Host (numpy): build level plans — per-level gather index arrays, window/chunk
    structure (uniform across cores for SPMD), initial-value code counts, readout
    layout.
  - Device: per level: indirect-DMA gather of source rows from table T ->
    PE matmul vs on-chip one-hot selection matrices (segment-sum + transpose in
    one op) -> PE matmul vs extended weights (aggregate @ W + counts @ (h0@W)
    + indeg*b) -> DMA -> 8-core AllGather into T's level block.
  - Readout: gather final rows of output nodes, per-graph sum via PE matmul
    against graph-indicator, per-graph max via DVE max-tree + PE transpose +
    free-axis reduce, tiny AllGather of per-core partials, combine.
"""

import sys

if "/opt/trn_rl_repo" not in sys.path:
    sys.path.insert(0, "/opt/trn_rl_repo")

import math

import numpy as np

# structural constants of the nn.Module (match reference)
B = 16   # graphs per batch
LF = 12  # forward topological levels
LB = 12  # backward topological levels
D = 64   # hidden dim
C = 8    # NeuronCores
P = 128  # SBUF partitions

# T table header rows
_ZROW = 0      # all-zero row (gather padding)
_H0 = 1        # 9 rows of initial-encoding table, indexed by 3*nt + nip
_MIROW = 11    # -float32.max row (readout max padding)
_HDR = 12


def _ceil(a, b):
    return -(-a // b)


_W = 32767  # dma_gather int16 source-window size (rows)


class _Lvl:
    """Static per-level plan (identical across cores)."""

    __slots__ = (
        "sweep", "l", "Lp", "tb", "ag0", "cnt0", "ww", "nwin", "nch", "ch0",
        "win_srcs", "extras", "segs", "acol0", "ccol0", "stg0", "A_pad",
    )


def _two_hop(gpos, idxA_list, idxC_list, seg_meta):
    """Build phase-A (per source window) + phase-C (permutation) index arrays.

    gpos: [C, S] int64 final-order T positions (-1 = padding slot).
    Appends int16 wrapped index blocks to idxA_list / idxC_list and returns
    (segs, A_pad): segs = [(window_base_row, n_slots)], A_pad = staging rows.
    Uniform across cores; per-core shortfalls padded with index 0.
    """
    Csz, S = gpos.shape
    sw = np.where(gpos >= 0, gpos // _W, -1)
    sws = sorted(set(sw[sw >= 0].tolist()))
    segs = []
    posA = np.zeros((Csz, S), np.int64)  # final slot -> phase-A position
    base = 0
    for s in sws:
        cnts = [(sw[c] == s).sum() for c in range(Csz)]
        n = _ceil(max(cnts), 128) * 128 if max(cnts) else 0
        if n == 0:
            continue
        for c in range(Csz):
            m = sw[c] == s
            posA[c, m] = base + np.arange(int(cnts[c]))
        segs.append((s * _W, n))
        base += n
    A_pad = base
    # idxA: per phase-A slot, offset within its window (0 for padding)
    idxA = np.zeros((Csz, max(A_pad, 128)), np.int16)
    for c in range(Csz):
        m = sw[c] >= 0
        idxA[c, posA[c, m]] = (gpos[c, m] % _W).astype(np.int16)
    A_pad = max(A_pad, 128)
    if not segs:
        segs = [(0, 128)]
    # idxC: final slot -> phase-A position (0 for padding)
    idxC = np.zeros((Csz, S), np.int16)
    for c in range(Csz):
        m = sw[c] >= 0
        idxC[c, m] = posA[c, m].astype(np.int16)
    # wrap into [128, n/16] layout, replicated into all 8 channel groups
    def wrap(a):
        n = a.shape[1]
        w = a.reshape(Csz, n // 16, 16).transpose(0, 2, 1)  # [C, 16, n/16]
        return np.ascontiguousarray(np.tile(w, (1, 8, 1)))  # [C, 128, n/16]
    idxA_list.append(wrap(idxA))
    idxC_list.append(wrap(idxC))
    seg_meta.append((segs, A_pad))
    return segs, A_pad


def _preprocess(node_type, num_inverted_predecessors, edge_index,
                forward_level, backward_level, batch,
                W_enc, b_enc, W_f, b_f, W_b, b_b):
    N = int(node_type.shape[0])
    nt = np.asarray(node_type).astype(np.int64)
    nip = np.asarray(num_inverted_predecessors).astype(np.int64)
    fl = np.asarray(forward_level).astype(np.int64)
    bl = np.asarray(backward_level).astype(np.int64)
    bt = np.asarray(batch).astype(np.int64)
    src = np.asarray(edge_index[0]).astype(np.int64)
    dst = np.asarray(edge_index[1]).astype(np.int64)
    code = nt * 3 + nip  # in [0, 9)

    # ---------------- node positions in T ----------------
    posf = np.full(N, -1, np.int64)
    posb = np.full(N, -1, np.int64)
    coref = np.zeros(N, np.int64)
    rankf = np.zeros(N, np.int64)
    coreb = np.zeros(N, np.int64)
    rankb = np.zeros(N, np.int64)

    tbase = _HDR
    lvl_meta = []  # (sweep, l, Lp, tbase)
    for sweep, lv, pos, core, rank, nl in (
        ("f", fl, posf, coref, rankf, LF),
        ("b", bl, posb, coreb, rankb, LB),
    ):
        for l in range(1, nl):
            idx = np.flatnonzero(lv == l)
            n_l = idx.size
            if n_l == 0:
                lvl_meta.append((sweep, l, 0, tbase))
                continue
            Lp = _ceil(_ceil(n_l, C), P) * P
            c = np.arange(n_l) % C
            r = np.arange(n_l) // C
            pos[idx] = tbase + c * Lp + r
            core[idx] = c
            rank[idx] = r
            lvl_meta.append((sweep, l, Lp, tbase))
            tbase += C * Lp
    T_ROWS = tbase

    # ---------------- per-level edge structures ----------------
    plans = []
    cdst_blocks = []   # [C, P, nch] float32
    cnts_blocks = []   # [C, 10, Lp] float32
    idxA_list = []     # [C, 128, n/16] int16 per level
    idxC_list = []
    seg_meta = []
    acol0 = ccol0 = stg0 = 0
    cnt0 = 0
    ch0 = 0
    fl_dst = fl[dst]
    bl_src = bl[src]
    for (sweep, l, Lp, tb) in lvl_meta:
        if Lp == 0:
            continue
        pl = _Lvl()
        pl.sweep, pl.l, pl.Lp, pl.tb = sweep, l, Lp, tb
        pl.ag0 = cnt0   # agin rows share the Lp-prefix-sum layout
        pl.cnt0 = cnt0
        if sweep == "f":
            em = fl_dst == l
            un = dst[em]
            dn = src[em]
            ucore = coref[un]
            urank = rankf[un]
            gat = (fl[dn] >= 1) & (fl[dn] < l)
            gpos_all = posf[dn]
        else:
            em = bl_src == l
            un = src[em]
            dn = dst[em]
            ucore = coreb[un]
            urank = rankb[un]
            upd_b = (bl[dn] >= 1) & (bl[dn] < l)
            upd_f = (~upd_b) & (fl[dn] >= 1)
            gat = upd_b | upd_f
            gpos_all = np.where(upd_b, posb[dn], posf[dn])

        # counts: initial-valued sources by code, plus total indegree (bias)
        cnt = np.zeros((C, Lp, 10), np.float32)
        i0 = ~gat
        np.add.at(cnt, (ucore[i0], urank[i0], code[dn[i0]]), 1.0)
        np.add.at(cnt, (ucore, urank, 9), 1.0)
        cnts_blocks.append(np.ascontiguousarray(cnt.transpose(0, 2, 1)))
        cnt0 += Lp

        # gather slots, grouped per destination-rank window, padded uniform
        gc = ucore[gat]
        gr = urank[gat]
        gp = gpos_all[gat]
        if gp.size == 0:
            pl.ww, pl.nwin, pl.nch, pl.ch0 = 128, 0, 0, ch0
            pl.win_srcs, pl.extras = [], []
            pl.segs, pl.acol0, pl.ccol0, pl.stg0, pl.A_pad = [], 0, 0, 0, 0
            plans.append(pl)
            continue
        nslot_c = np.bincount(gc, minlength=C)
        dens = nslot_c.max() / Lp
        ww = min(4, max(1, _ceil(96, int(128 * dens) + 1))) * 128
        nwin = _ceil(Lp, ww)
        w_e = gr // ww
        cw = np.zeros((C, nwin), np.int64)
        np.add.at(cw, (gc, w_e), 1)
        Pw = cw.max(axis=0)
        S_total = int(Pw.sum())
        nch = _ceil(S_total, P)
        S_pad = nch * P
        w_slot = np.repeat(np.arange(nwin), Pw)
        w_slot = np.concatenate(
            [w_slot, np.full(S_pad - S_total, max(0, nwin - 1), np.int64)])
        win_base = np.concatenate([[0], np.cumsum(Pw)])[:-1]

        r_sl = np.full((C, S_pad), -1, np.int64)
        g_sl = np.zeros((C, S_pad), np.int64)
        order = np.lexsort((gr, w_e, gc))
        gc_o, gr_o, gp_o, w_o = gc[order], gr[order], gp[order], w_e[order]
        grp = gc_o * nwin + w_o
        counts = np.bincount(grp, minlength=C * nwin)
        gstart = np.concatenate([[0], np.cumsum(counts)])[:-1]
        seq = np.arange(gc_o.size) - gstart[grp]
        slotpos = win_base[w_o] + seq
        r_sl[gc_o, slotpos] = gr_o
        g_sl[gc_o, slotpos] = gp_o

        ch_wfirst = w_slot[::P]  # [nch]
        slot_chunk = np.arange(S_pad) // P
        cd = r_sl.astype(np.float64) - (ww * ch_wfirst[slot_chunk])[None, :]
        cd[r_sl < 0] = -1.0

        cdst_blocks.append(np.ascontiguousarray(
            cd.reshape(C, nch, P).transpose(0, 2, 1)).astype(np.float32))
        gpos_lin = np.where(r_sl >= 0, g_sl, -1)
        segs, A_pad = _two_hop(gpos_lin, idxA_list, idxC_list, seg_meta)
        pl.segs, pl.A_pad = segs, A_pad
        pl.acol0, pl.ccol0, pl.stg0 = acol0, ccol0, stg0
        acol0 += idxA_list[-1].shape[2]
        ccol0 += idxC_list[-1].shape[2]
        stg0 += A_pad

        win_srcs = [[] for _ in range(nwin)]
        extras = []
        for j in range(nch):
            ws = np.unique(w_slot[j * P:(j + 1) * P])
            wf = int(ch_wfirst[j])
            for w in ws:
                k = int(w) - wf
                win_srcs[int(w)].append((j, k))
                if k >= 1:
                    extras.append((j, k))
        pl.ww, pl.nwin, pl.nch, pl.ch0 = ww, nwin, nch, ch0
        pl.win_srcs, pl.extras = win_srcs, extras
        ch0 += nch
        plans.append(pl)

    TOTCH = max(1, ch0)
    CNT_TOT = cnt0
    cdst_all = (np.concatenate(cdst_blocks, axis=2) if cdst_blocks
                else np.full((C, P, 1), -1.0, np.float32))
    if cdst_all.shape[2] < TOTCH:  # pad to TOTCH (degenerate case)
        pad = TOTCH - cdst_all.shape[2]
        cdst_all = np.concatenate(
            [cdst_all, np.full((C, P, pad), -1.0, np.float32)], axis=2)
    cnts_all = np.concatenate(cnts_blocks, axis=2)

    # ---------------- readout layout ----------------
    onodes = np.flatnonzero(nt == 1)
    og = bt[onodes]
    fpos = np.where(bl[onodes] >= 1, posb[onodes],
                    np.where(fl[onodes] >= 1, posf[onodes],
                             _H0 + code[onodes]))
    graph_chunks = []
    kg_list = []
    for g in range(B):
        n_g = int((og == g).sum())
        kg = max(1, _ceil(_ceil(max(n_g, 1), C), P))
        kg_list.append(kg)
    NRCH = int(np.sum(kg_list))
    c0s = np.concatenate([[0], np.cumsum(kg_list)])[:-1]
    roff = np.full((C, P, NRCH), -1, np.int64)
    rgid = np.full((C, P, NRCH), -1.0, np.float32)
    for g in range(B):
        m = og == g
        npos = fpos[m]
        n_g = npos.size
        graph_chunks.append((int(c0s[g]), kg_list[g]))
        if n_g == 0:
            continue
        c = np.arange(n_g) % C
        sq = np.arange(n_g) // C
        j = sq // P
        p = sq % P
        roff[c, p, int(c0s[g]) + j] = npos
        rgid[c, p, int(c0s[g]) + j] = g
    # padding readout slots gather the -inf row (neutral for max; rgid=-1
    # keeps them out of the sum)
    roff[roff < 0] = _MIROW
    # readout two-hop (linear slot index s = j*128 + p)
    roff_lin = np.ascontiguousarray(
        roff.transpose(0, 2, 1)).reshape(C, NRCH * P)
    ro_segs, ro_A = _two_hop(roff_lin, idxA_list, idxC_list, seg_meta)
    ro_acol0, ro_ccol0, ro_stg0 = acol0, ccol0, stg0
    acol0 += idxA_list[-1].shape[2]
    ccol0 += idxC_list[-1].shape[2]
    stg0 += ro_A

    # ---------------- weight-derived constants ----------------
    W_enc = np.asarray(W_enc, np.float32)
    b_enc = np.asarray(b_enc, np.float32)
    W_f = np.asarray(W_f, np.float32)
    b_f = np.asarray(b_f, np.float32)
    W_b = np.asarray(W_b, np.float32)
    b_b = np.asarray(b_b, np.float32)
    h0_tab = np.zeros((9, D), np.float32)
    for cc in range(9):
        h0_tab[cc] = (cc // 3) * W_enc[0] + (cc % 3) * W_enc[1] + b_enc
    tab = np.zeros((_HDR, D), np.float32)
    tab[_H0:_H0 + 9] = h0_tab
    tab[_MIROW] = np.finfo(np.float32).min
    wf_ext = np.concatenate([W_f, h0_tab @ W_f, b_f[None, :]], axis=0)
    wb_ext = np.concatenate([W_b, h0_tab @ W_b, b_b[None, :]], axis=0)
    wext = np.ascontiguousarray(np.concatenate([wf_ext, wb_ext], axis=1))

    iota512 = np.ascontiguousarray(
        np.tile(np.arange(512, dtype=np.float32), (P, 1)))
    iota16 = np.ascontiguousarray(
        np.tile(np.arange(16, dtype=np.float32), (P, 1)))

    idxA_all = np.concatenate(idxA_list, axis=2)
    idxC_all = np.concatenate(idxC_list, axis=2)
    meta = dict(
        plans=plans, graph_chunks=graph_chunks,
        T_ROWS=T_ROWS, AG_ROWS=max(1, CNT_TOT), TOTCH=TOTCH,
        CNT_TOT=max(1, CNT_TOT), NRCH=NRCH,
        AW=idxA_all.shape[2], CW=idxC_all.shape[2], STG=stg0,
        ro=dict(segs=ro_segs, A_pad=ro_A, acol0=ro_acol0,
                ccol0=ro_ccol0, stg0=ro_stg0),
    )
    arrays = dict(
        tab=tab, wext=wext, iota512=iota512, iota16=iota16,
        idxA=idxA_all, idxC=idxC_all, cdst=cdst_all, cnts=cnts_all,
        rgid=rgid,
    )
    return meta, arrays


# ---------------------------------------------------------------------------
# pure-numpy execution of the plan (host self-check / debugging)
# ---------------------------------------------------------------------------

def _gather_two_hop(T, arrays, c, segs, A_pad, acol0, ccol0, n_slots):
    """numpy reference of the device two-hop gather; returns [n_slots, D]."""
    idxA = arrays["idxA"][c][0:16, :]
    idxC = arrays["idxC"][c][0:16, :]
    stg = np.zeros((A_pad, D), T.dtype)
    base = 0
    for (swb, n) in segs:
        cols = slice(acol0 + base // 16, acol0 + (base + n) // 16)
        off = idxA[:, cols].T.reshape(-1)[:n].astype(np.int64)
        stg[base:base + n] = T[swb + off]
        base += n
    perm = idxC[:, ccol0:ccol0 + n_slots // 16].T.reshape(-1).astype(np.int64)
    return stg[perm]


def _simulate_plan(meta, arrays, return_T=False):
    T = np.zeros((meta["T_ROWS"], D), np.float32)
    T[0:_HDR] = arrays["tab"]
    wext = arrays["wext"]
    for pl in meta["plans"]:
        wmat = wext[:, 0:D] if pl.sweep == "f" else wext[:, D:2 * D]
        blocks = []
        for c in range(C):
            lhs = np.zeros((74, pl.Lp), np.float32)
            lhs[64:74] = arrays["cnts"][c, :, pl.cnt0:pl.cnt0 + pl.Lp]
            if pl.nch > 0:
                cdv = arrays["cdst"][c][:, pl.ch0:pl.ch0 + pl.nch]
                G_lin = _gather_two_hop(T, arrays, c, pl.segs, pl.A_pad,
                                        pl.acol0, pl.ccol0, pl.nch * P)
                G = G_lin.reshape(pl.nch, P, D).transpose(1, 0, 2)
                written = np.zeros(pl.nwin, bool)
                aggT = np.zeros((64, pl.Lp), np.float32)
                for w in range(pl.nwin):
                    width = min(pl.ww, pl.Lp - w * pl.ww)
                    for (j, k) in pl.win_srcs[w]:
                        S = (cdv[:, j:j + 1] ==
                             (np.arange(width) + k * pl.ww)[None, :])
                        aggT[:, w * pl.ww:w * pl.ww + width] += (
                            G[:, j, :].T @ S.astype(np.float32))
                        written[w] = True
                for w in range(pl.nwin):
                    if not written[w]:
                        width = min(pl.ww, pl.Lp - w * pl.ww)
                        aggT[:, w * pl.ww:w * pl.ww + width] = 0.0
                lhs[0:64] = aggT
            blocks.append(lhs.T @ wmat)  # [Lp, D]
        T[pl.tb:pl.tb + C * pl.Lp] = np.concatenate(blocks, axis=0)
    # readout
    maxp = np.full((B, D), np.finfo(np.float32).min, np.float32)
    sump = np.zeros((B, D), np.float32)
    ro = meta["ro"]
    for c in range(C):
        R_lin = _gather_two_hop(T, arrays, c, ro["segs"], ro["A_pad"],
                                ro["acol0"], ro["ccol0"],
                                meta["NRCH"] * P)
        R = R_lin.reshape(meta["NRCH"], P, D).transpose(1, 0, 2)
        gid = arrays["rgid"][c]           # [P, NRCH]
        for g, (c0, kg) in enumerate(meta["graph_chunks"]):
            sl = R[:, c0:c0 + kg, :]
            maxp[g] = np.maximum(maxp[g], sl.max(axis=(0, 1)))
            msk = (gid[:, c0:c0 + kg] == g).astype(np.float32)
            sump[g] += np.einsum("pk,pkd->d", msk, sl)
    out = np.concatenate([maxp, sump], axis=1)
    return (out, T) if return_T else out


# ---------------------------------------------------------------------------
# Bass program
# ---------------------------------------------------------------------------

def _build(meta):
    import concourse.bass as bass
    import concourse.mybir as mybir
    from concourse import bacc, tile
    from concourse.masks import make_identity

    f32 = mybir.dt.float32
    i16 = mybir.dt.int16
    AX = mybir.AxisListType
    OP = mybir.AluOpType

    TOTCH, CNT_TOT, NRCH = meta["TOTCH"], meta["CNT_TOT"], meta["NRCH"]
    AW, CW, STG = meta["AW"], meta["CW"], meta["STG"]

    nc = bacc.Bacc(None, num_devices=C)
    tab_x = nc.dram_tensor("tab", [_HDR, D], f32, kind="ExternalInput")
    wext_x = nc.dram_tensor("wext", [74, 2 * D], f32, kind="ExternalInput")
    iota_x = nc.dram_tensor("iota", [P, 512], f32, kind="ExternalInput")
    io16_x = nc.dram_tensor("iota16", [P, 16], f32, kind="ExternalInput")
    idxA_x = nc.dram_tensor("idxA", [P, AW], i16, kind="ExternalInput")
    idxC_x = nc.dram_tensor("idxC", [P, CW], i16, kind="ExternalInput")
    cdst_x = nc.dram_tensor("cdst", [P, TOTCH], f32, kind="ExternalInput")
    cnts_x = nc.dram_tensor("cnts", [10, CNT_TOT], f32, kind="ExternalInput")
    rgid_x = nc.dram_tensor("rgid", [P, NRCH], f32, kind="ExternalInput")
    out_x = nc.dram_tensor("out", [B, 2 * D], f32, kind="ExternalOutput")

    T = nc.dram_tensor("T", [meta["T_ROWS"], D], f32, addr_space="Shared")
    import os
    dump_T = bool(os.environ.get("KERNEL_DUMP_T"))
    tdump_x = (nc.dram_tensor("tdump", [meta["T_ROWS"], D], f32,
                              kind="ExternalOutput") if dump_T else None)
    agin = nc.dram_tensor("agin", [meta["AG_ROWS"], D], f32)
    stg_t = nc.dram_tensor("stg", [max(STG, 128), D], f32)
    prin = nc.dram_tensor("prin", [2 * B, D], f32)
    prout = nc.dram_tensor("prout", [C * 2 * B, D], f32, addr_space="Shared")

    rg = [list(range(C))]

    with tile.TileContext(nc) as tc:
        with (
            tc.tile_pool(name="const", bufs=1) as constp,
            tc.tile_pool(name="g", bufs=2) as gpool,
            tc.tile_pool(name="ro", bufs=1) as ropool,
            tc.tile_pool(name="sall", bufs=2) as sallp,
            tc.tile_pool(name="sx", bufs=4) as sxp,
            tc.tile_pool(name="lhs", bufs=2) as lhsp,
            tc.tile_pool(name="msg", bufs=2) as msgp,
            tc.tile_pool(name="small", bufs=2) as smp,
            tc.tile_pool(name="psA", bufs=2, space="PSUM") as psA,
            tc.tile_pool(name="psB", bufs=2, space="PSUM") as psB,
            tc.tile_pool(name="psC", bufs=2, space="PSUM") as psC,
            tc.tile_pool(name="psR", bufs=1, space="PSUM") as psR,
        ):
            # ---------------- setup ----------------
            stg = constp.tile([_HDR, D], f32)
            nc.sync.dma_start(out=stg[:, :], in_=tab_x[:, :])
            nc.sync.dma_start(out=T[0:_HDR, :], in_=stg[:, :])
            wt = constp.tile([74, 2 * D], f32)
            nc.sync.dma_start(out=wt[:, :], in_=wext_x[:, :])
            iot = constp.tile([P, 512], f32)
            nc.sync.dma_start(out=iot[:, :], in_=iota_x[:, :])
            io16 = constp.tile([P, 16], f32)
            nc.sync.dma_start(out=io16[:, :], in_=io16_x[:, :])
            cdstt = constp.tile([P, TOTCH], f32)
            nc.sync.dma_start(out=cdstt[:, :], in_=cdst_x[:, :])
            idxAt = constp.tile([P, AW], i16)
            nc.sync.dma_start(out=idxAt[:, :], in_=idxA_x[:, :])
            idxCt = constp.tile([P, CW], i16)
            nc.sync.dma_start(out=idxCt[:, :], in_=idxC_x[:, :])
            rgidt = constp.tile([P, NRCH], f32)
            nc.sync.dma_start(out=rgidt[:, :], in_=rgid_x[:, :])
            ident = constp.tile([P, P], f32)
            make_identity(nc, ident[:, :])

            def two_hop_gather(segs, A_pad, acol0, ccol0, stg0, n_slots,
                               gpool_, tagA, tagC, frontier):
                """Emit phase A (window gathers) -> staging -> phase C."""
                nchA = A_pad // P
                GA = gpool_.tile([P, nchA * D], f32, tag=tagA)
                base = 0
                for (swb, n) in segs:
                    wrows = min(_W, frontier - swb)
                    k = n // P
                    j0 = base // P
                    nc.gpsimd.dma_gather(
                        out_ap=GA[:, j0 * D:(j0 + k) * D].rearrange(
                            "p (j f) -> p j f", f=D),
                        in_ap=T[swb:swb + wrows, :],
                        idxs_ap=idxAt[:, acol0 + base // 16:
                                      acol0 + (base + n) // 16],
                        num_idxs=n, num_idxs_reg=n, elem_size=D,
                        single_packet=False)
                    base += n
                nc.sync.dma_start(
                    out=stg_t[stg0:stg0 + A_pad, :].rearrange(
                        "(j p) f -> p j f", p=P),
                    in_=GA[:, :].rearrange("p (j f) -> p j f", f=D))
                k = n_slots // P
                G = gpool_.tile([P, k * D], f32, tag=tagC)
                nc.gpsimd.dma_gather(
                    out_ap=G[:, :].rearrange("p (j f) -> p j f", f=D),
                    in_ap=stg_t[stg0:stg0 + A_pad, :],
                    idxs_ap=idxCt[:, ccol0:ccol0 + n_slots // 16],
                    num_idxs=n_slots, num_idxs_reg=n_slots, elem_size=D,
                    single_packet=False)
                return G

            # ---------------- level sweeps ----------------
            max_lv = int(os.environ.get("KERNEL_MAX_LEVELS", "99"))
            skip_ro = bool(os.environ.get("KERNEL_SKIP_READOUT"))
            for pl in meta["plans"][:max_lv]:
                Lp, ww, nch, nwin = pl.Lp, pl.ww, pl.nch, pl.nwin
                nw2 = Lp // P
                lhs = lhsp.tile([74, Lp], f32)
                nc.sync.dma_start(
                    out=lhs[64:74, :],
                    in_=cnts_x[:, pl.cnt0:pl.cnt0 + Lp])
                if nch > 0:
                    G = two_hop_gather(pl.segs, pl.A_pad, pl.acol0, pl.ccol0,
                                       pl.stg0, nch * P, gpool, "GA", "G",
                                       pl.tb)
                    # bulk k=0 selection matrices
                    Sall = sallp.tile([P, nch * ww], f32)
                    ia = iot[:, 0:ww]
                    iota_rep = bass.AP(
                        ia.tensor, ia.offset, [ia.ap[0], [0, nch], [1, ww]])
                    cdb = cdstt[:, pl.ch0:pl.ch0 + nch].to_broadcast(
                        [P, nch, ww])
                    nc.vector.tensor_tensor(
                        out=Sall[:, :].rearrange("p (a b) -> p a b", a=nch),
                        in0=iota_rep, in1=cdb, op=OP.is_equal)
                    extS = {}
                    for (j, k) in pl.extras:
                        S2 = sxp.tile([P, ww], f32)
                        nc.vector.tensor_scalar(
                            S2[:, :], iot[:, 0:ww],
                            cdstt[:, pl.ch0 + j:pl.ch0 + j + 1],
                            float(-k * ww), OP.subtract, OP.is_equal)
                        extS[(j, k)] = S2
                    for w in range(nwin):
                        width = min(ww, Lp - w * ww)
                        srcs = pl.win_srcs[w]
                        if not srcs:
                            nc.vector.memset(
                                lhs[0:64, w * ww:w * ww + width], 0.0)
                            continue
                        ps = psA.tile([64, ww], f32)
                        for i, (j, k) in enumerate(srcs):
                            if k == 0:
                                S_ap = Sall[:, j * ww:j * ww + width]
                            else:
                                S_ap = extS[(j, k)][:, 0:width]
                            nc.tensor.matmul(
                                out=ps[:, 0:width],
                                lhsT=G[:, j * D:(j + 1) * D],
                                rhs=S_ap,
                                start=(i == 0), stop=(i == len(srcs) - 1))
                        nc.vector.tensor_copy(
                            out=lhs[0:64, w * ww:w * ww + width],
                            in_=ps[:, 0:width])
                else:
                    nc.vector.memset(lhs[0:64, :], 0.0)

                msg = msgp.tile([P, nw2 * D], f32)
                wcol = 0 if pl.sweep == "f" else D
                for w2 in range(nw2):
                    ps2 = psB.tile([P, D], f32)
                    nc.tensor.matmul(
                        out=ps2[:, :],
                        lhsT=lhs[:, w2 * P:(w2 + 1) * P],
                        rhs=wt[:, wcol:wcol + D],
                        start=True, stop=True)
                    nc.vector.tensor_copy(
                        out=msg[:, w2 * D:(w2 + 1) * D], in_=ps2[:, :])
                nc.sync.dma_start(
                    out=agin[pl.ag0:pl.ag0 + Lp, :].rearrange(
                        "(j p) f -> p j f", p=P),
                    in_=msg[:, :].rearrange("p (j f) -> p j f", f=D))
                nc.gpsimd.collective_compute(
                    "AllGather", OP.bypass, replica_groups=rg,
                    ins=[agin[pl.ag0:pl.ag0 + Lp, :]],
                    outs=[T[pl.tb:pl.tb + C * Lp, :]])

            # ---------------- readout ----------------
            if skip_ro:
                zo = smp.tile([B, 2 * D], f32, tag="outt")
                nc.vector.memset(zo[:, :], 0.0)
                nc.sync.dma_start(out=out_x[:, :], in_=zo[:, :])
                if dump_T:
                    nc.sync.dma_start(out=tdump_x[:, :], in_=T[:, :])
            if not skip_ro:
                _emit_readout(
                    nc, bass, mybir, meta, tc, constp, ropool, sallp, smp,
                    psB, psC, psR, rgidt, io16, ident, stg_t, idxAt, idxCt,
                    agin, prin, prout, out_x, tdump_x, dump_T, two_hop_gather,
                    rg, T)
    nc.compile()
    return nc


def _emit_readout(nc, bass, mybir, meta, tc, constp, ropool, sallp, smp,
                  psB, psC, psR, rgidt, io16, ident, stg_t, idxAt, idxCt,
                  agin, prin, prout, out_x, tdump_x, dump_T, two_hop_gather,
                  rg, T):
    f32 = mybir.dt.float32
    AX = mybir.AxisListType
    OP = mybir.AluOpType
    NRCH = meta["NRCH"]
    import os
    stage = int(os.environ.get("KERNEL_RO_STAGE", "9"))
    if True:
        if True:
            ro = meta["ro"]
            R = two_hop_gather(ro["segs"], ro["A_pad"], ro["acol0"],
                               ro["ccol0"], ro["stg0"], NRCH * P,
                               ropool, "RA", "R", meta["T_ROWS"])
            if stage <= 1:
                zo = smp.tile([B, 2 * D], f32, tag="outt")
                nc.vector.tensor_copy(out=zo[:, :], in_=R[0:B, 0:2 * D])
                nc.sync.dma_start(out=out_x[:, :], in_=zo[:, :])
                return
            S16 = sallp.tile([P, NRCH * 16], f32, tag="s16")
            i16 = io16[:, 0:16]
            i16_rep = bass.AP(
                i16.tensor, i16.offset, [i16.ap[0], [0, NRCH], [1, 16]])
            rgb = rgidt[:, :].to_broadcast([P, NRCH, 16])
            nc.vector.tensor_tensor(
                out=S16[:, :].rearrange("p (a b) -> p a b", a=NRCH),
                in0=i16_rep, in1=rgb, op=OP.is_equal)
            ps_sum = psR.tile([B, D], f32)
            for j in range(NRCH):
                nc.tensor.matmul(
                    out=ps_sum[:, :],
                    lhsT=S16[:, j * 16:(j + 1) * 16],
                    rhs=R[:, j * D:(j + 1) * D],
                    start=(j == 0), stop=(j == NRCH - 1))
            if stage <= 2:
                zo = smp.tile([B, 2 * D], f32, tag="outt")
                nc.vector.tensor_copy(out=zo[:, 0:D], in_=ps_sum[:, :])
                nc.vector.memset(zo[:, D:2 * D], 0.0)
                nc.sync.dma_start(out=out_x[:, :], in_=zo[:, :])
                return
            maxT = constp.tile([64, B], f32)
            for g, (c0, kg) in enumerate(meta["graph_chunks"]):
                if kg == 1:
                    mx_ap = R[:, c0 * D:(c0 + 1) * D]
                else:
                    mx = smp.tile([P, D], f32, tag="mx")
                    nc.vector.tensor_tensor(
                        out=mx[:, :], in0=R[:, c0 * D:(c0 + 1) * D],
                        in1=R[:, (c0 + 1) * D:(c0 + 2) * D], op=OP.max)
                    for q in range(2, kg):
                        nc.vector.tensor_tensor(
                            out=mx[:, :], in0=mx[:, :],
                            in1=R[:, (c0 + q) * D:(c0 + q + 1) * D],
                            op=OP.max)
                    mx_ap = mx[:, :]
                pst = psC.tile([64, P], f32, tag="pst")
                nc.tensor.transpose(
                    out=pst[:, :], in_=mx_ap, identity=ident[:, :])
                nc.vector.reduce_max(
                    out=maxT[:, g:g + 1], in_=pst[:, :], axis=AX.X)
            if stage <= 3:
                zo = smp.tile([B, 2 * D], f32, tag="outt")
                nc.vector.memset(zo[:, :], 0.0)
                nc.sync.dma_start(out=out_x[:, :], in_=zo[:, :])
                return
            psmx = psC.tile([B, 64], f32, tag="pst")
            nc.tensor.transpose(
                out=psmx[:, :], in_=maxT[:, :], identity=ident[0:64, 0:64])
            pr = smp.tile([B, 2 * D], f32, tag="pr")
            nc.vector.tensor_copy(out=pr[:, 0:D], in_=psmx[:, :])
            nc.vector.tensor_copy(out=pr[:, D:2 * D], in_=ps_sum[:, :])
            nc.sync.dma_start(
                out=prin[:, :].rearrange("(h g) f -> g h f", g=B),
                in_=pr[:, :].rearrange("g (h f) -> g h f", h=2))
            nc.gpsimd.collective_compute(
                "AllGather", OP.bypass, replica_groups=rg,
                ins=[prin[:, :]], outs=[prout[:, :]])
            if stage <= 4:
                zo = smp.tile([B, 2 * D], f32, tag="outt")
                nc.vector.memset(zo[:, :], 0.0)
                nc.sync.dma_start(out=out_x[:, :], in_=zo[:, :])
                return
            pr3 = prout[:, :].rearrange("(r gg) f -> gg r f", r=C)
            mx8 = smp.tile([B, C * D], f32, tag="mx8")
            sm8 = smp.tile([B, C * D], f32, tag="sm8")
            nc.sync.dma_start(
                out=mx8[:, :].rearrange("g (r f) -> g r f", f=D),
                in_=pr3[0:B])
            nc.sync.dma_start(
                out=sm8[:, :].rearrange("g (r f) -> g r f", f=D),
                in_=pr3[B:2 * B])
            outt = smp.tile([B, 2 * D], f32, tag="outt")
            t1m = smp.tile([B, 4 * D], f32, tag="t1m")
            t2m = smp.tile([B, 2 * D], f32, tag="t2m")
            nc.vector.tensor_tensor(
                out=t1m[:, :], in0=mx8[:, 0:4 * D], in1=mx8[:, 4 * D:8 * D],
                op=OP.max)
            nc.vector.tensor_tensor(
                out=t2m[:, :], in0=t1m[:, 0:2 * D], in1=t1m[:, 2 * D:4 * D],
                op=OP.max)
            nc.vector.tensor_tensor(
                out=outt[:, 0:D], in0=t2m[:, 0:D], in1=t2m[:, D:2 * D],
                op=OP.max)
            t1s = smp.tile([B, 4 * D], f32, tag="t1s")
            t2s = smp.tile([B, 2 * D], f32, tag="t2s")
            nc.vector.tensor_tensor(
                out=t1s[:, :], in0=sm8[:, 0:4 * D], in1=sm8[:, 4 * D:8 * D],
                op=OP.add)
            nc.vector.tensor_tensor(
                out=t2s[:, :], in0=t1s[:, 0:2 * D], in1=t1s[:, 2 * D:4 * D],
                op=OP.add)
            nc.vector.tensor_tensor(
                out=outt[:, D:2 * D], in0=t2s[:, 0:D], in1=t2s[:, D:2 * D],
                op=OP.add)
            nc.sync.dma_start(out=out_x[:, :], in_=outt[:, :])
            if dump_T:
                nc.sync.dma_start(out=tdump_x[:, :], in_=T[:, :])


def _in_maps(meta, arrays):
    maps = []
    for c in range(C):
        maps.append(dict(
            tab=arrays["tab"],
            wext=arrays["wext"],
            iota=arrays["iota512"],
            iota16=arrays["iota16"],
            idxA=np.ascontiguousarray(arrays["idxA"][c]),
            idxC=np.ascontiguousarray(arrays["idxC"][c]),
            cdst=np.ascontiguousarray(arrays["cdst"][c]),
            cnts=np.ascontiguousarray(arrays["cnts"][c]),
            rgid=np.ascontiguousarray(arrays["rgid"][c]),
        ))
    return maps


_LAST_RESULTS = None  # stash for test harness (exec time, trace)


def kernel(**inputs):
    global _LAST_RESULTS
    import os
    meta, arrays = _preprocess(**inputs)
    nc = _build(meta)
    from concourse.bass_utils import run_bass_kernel_spmd
    res = run_bass_kernel_spmd(nc, _in_maps(meta, arrays),
                               core_ids=list(range(C)),
                               trace=bool(os.environ.get("KERNEL_TRACE")))
    _LAST_RESULTS = res
    return np.asarray(res.results[0]["out"])



# revision 12
# speedup vs baseline: 1.3246x; 1.3246x over previous
"""DAG-GNN level-sweep kernel for Trainium2 (8 NeuronCores, Bass/Tile), v3.

Single-hop gather design:
  - T table in DRAM stores fp16 *pair rows*: one 256B row = two node states
    (64 fp16 each). Gathers use int16 pair indices over 32767-pair windows
    (~3.8 level blocks per window), so each per-level gather is one hop in
    phase-A order (grouped by source window, sorted by dest rank) - no
    staging round-trip, no permutation gather.
  - A host-shipped 0/1 parity mask zeroes the unwanted half of each gathered
    pair (one bulk DVE multiply). The doubled feature rows fold back in the
    message matmul against [W; W]-stacked weights.
  - Segment-sum via PE matmuls of gather chunks against bulk-built one-hot
    selection matrices (per dest window of 512, fresh-source pieces last so
    old work overlaps the per-level AllGather).
  - Per-level fp16 AllGather of pair rows (half the bytes of f32).
  - Readout: two-hop (phase A overlapped into the backward sweep, small
    permutation gather at the end), parity-masked, sum via one-hot matmul,
    max via DVE max-tree + PE transpose.
"""

import sys

if "/opt/trn_rl_repo" not in sys.path:
    sys.path.insert(0, "/opt/trn_rl_repo")

import numpy as np

# structural constants of the nn.Module (match reference)
B = 16   # graphs per batch
LF = 12  # forward topological levels
LB = 12  # backward topological levels
D = 64   # hidden dim
C = 8    # NeuronCores
P = 128  # SBUF partitions
WW = 512  # dest window width for selection matmuls (one PSUM bank fp32)
BIG = 30000.0  # -BIG fills the readout max-padding row (fp16-safe)

# T header pair rows
_ZP = 0        # all-zero pair (gather padding target in window 0)
_H0P = 1       # 9 pair rows of initial-encoding table, indexed by 3*nt+nip
_MIP = 10      # [-BIG]*128 pair (readout max padding)
_HDRP = 12

_W = 32767  # dma_gather int16 source-window size (pair rows)


def _ceil(a, b):
    return -(-a // b)


class _Lvl:
    """Static per-level plan (identical across cores)."""

    __slots__ = (
        "sweep", "l", "Lp", "nw2", "half", "pr", "tbp", "cnt0", "ag0",
        "wins", "A_pad", "nch", "a0", "pc0", "pq0", "pieces", "npieces",
        "wchunks",
    )


def _wrap_idx(a):
    """[C, n] int16 -> [C, 128, n/16] (16-partition wrap, replicated x8)."""
    Csz, n = a.shape
    w = a.reshape(Csz, n // 16, 16).transpose(0, 2, 1)
    return np.ascontiguousarray(np.tile(w, (1, 8, 1)))


def _preprocess(node_type, num_inverted_predecessors, edge_index,
                forward_level, backward_level, batch,
                W_enc, b_enc, W_f, b_f, W_b, b_b):
    N = int(node_type.shape[0])
    nt = np.asarray(node_type).astype(np.int64)
    nip = np.asarray(num_inverted_predecessors).astype(np.int64)
    fl = np.asarray(forward_level).astype(np.int64)
    bl = np.asarray(backward_level).astype(np.int64)
    bt = np.asarray(batch).astype(np.int64)
    src = np.asarray(edge_index[0]).astype(np.int64)
    dst = np.asarray(edge_index[1]).astype(np.int64)
    code = nt * 3 + nip  # in [0, 9)

    # ---------------- node positions in pair-packed T ----------------
    pairf = np.full(N, -1, np.int64)
    parf = np.zeros(N, np.int64)
    pairb = np.full(N, -1, np.int64)
    parb = np.zeros(N, np.int64)
    coref = np.zeros(N, np.int64)
    rankf = np.zeros(N, np.int64)
    coreb = np.zeros(N, np.int64)
    rankb = np.zeros(N, np.int64)

    tbp = _HDRP
    lvl_meta = []  # (sweep, l, Lp, tbp)
    for sweep, lv, pair, par, core, rank, nl in (
        ("f", fl, pairf, parf, coref, rankf, LF),
        ("b", bl, pairb, parb, coreb, rankb, LB),
    ):
        for l in range(1, nl):
            idx = np.flatnonzero(lv == l)
            n_l = idx.size
            if n_l == 0:
                lvl_meta.append((sweep, l, 0, tbp))
                continue
            Lp = _ceil(_ceil(n_l, C), 256) * 256
            half = Lp // 2
            c = np.arange(n_l) % C
            r = np.arange(n_l) // C
            pair[idx] = tbp + c * half + (r % half)
            par[idx] = r // half
            core[idx] = c
            rank[idx] = r
            lvl_meta.append((sweep, l, Lp, tbp))
            tbp += C * half
    TP = tbp  # total T pair rows

    # ---------------- per-level plans ----------------
    plans = []
    idxA_list = []    # per-window wrapped idx blocks [C,128,n/16]
    pcd_list = []     # [C, P, npieces] f32 (cast f16 at ship)
    par2_list = []    # [C, P, 2*nch]
    cnts_list = []    # [C, 10, Lp]
    a0 = pc0 = pq0 = 0
    cnt0 = 0
    ag0 = 0
    fl_dst = fl[dst]
    bl_src = bl[src]
    prev_tb = _HDRP  # base pair of the block written one step earlier
    for (sweep, l, Lp, tb) in lvl_meta:
        if Lp == 0:
            prev_tb = tb
            continue
        pl = _Lvl()
        pl.sweep, pl.l, pl.Lp, pl.tbp = sweep, l, Lp, tb
        pl.nw2 = Lp // P
        pl.half = Lp // 2
        pl.pr = pl.half
        pl.cnt0 = cnt0
        pl.ag0 = ag0
        if sweep == "f":
            em = fl_dst == l
            un = dst[em]
            dn = src[em]
            ucore = coref[un]
            urank = rankf[un]
            gat = (fl[dn] >= 1) & (fl[dn] < l)
            gpair_all = pairf[dn]
            gpar_all = parf[dn]
        else:
            em = bl_src == l
            un = src[em]
            dn = dst[em]
            ucore = coreb[un]
            urank = rankb[un]
            upd_b = (bl[dn] >= 1) & (bl[dn] < l)
            upd_f = (~upd_b) & (fl[dn] >= 1)
            gat = upd_b | upd_f
            gpair_all = np.where(upd_b, pairb[dn], pairf[dn])
            gpar_all = np.where(upd_b, parb[dn], parf[dn])

        # counts: initial-valued sources by code, plus total indegree (bias)
        cnt = np.zeros((C, Lp, 10), np.float32)
        i0 = ~gat
        np.add.at(cnt, (ucore[i0], urank[i0], code[dn[i0]]), 1.0)
        np.add.at(cnt, (ucore, urank, 9), 1.0)
        cnts_list.append(np.ascontiguousarray(cnt.transpose(0, 2, 1)))
        cnt0 += Lp
        ag0 += pl.pr

        gc = ucore[gat]
        gr = urank[gat]
        gp = gpair_all[gat]
        gq = gpar_all[gat]
        if gp.size == 0:
            pl.wins, pl.A_pad, pl.nch, pl.npieces = [], 0, 0, 0
            pl.a0, pl.pc0, pl.pq0 = a0, pc0, pq0
            pl.pieces, pl.wchunks = [], []
            plans.append(pl)
            prev_tb = tb
            continue

        # windows (fixed 32767-pair grid), per-core slots sorted by dest rank
        sw = gp // _W
        sws = sorted(set(sw.tolist()))
        wins = []          # (swb, wrows, n_slots, fresh)
        idx16 = []         # per-core concatenated int16 idx
        slot_gr = np.full((C, 0), -1, np.int64)
        gr_cols = []
        par_cols = []
        order = np.lexsort((gr, sw, gc))
        gc_o, gr_o, gp_o, gq_o, sw_o = (
            gc[order], gr[order], gp[order], gq[order], sw[order])
        idxA_core = [[] for _ in range(C)]
        for s in sws:
            swb = s * _W
            m = sw_o == s
            cnts_c = np.bincount(gc_o[m], minlength=C)
            n_s = max(128, _ceil(int(cnts_c.max()), 128) * 128)
            wrows = int(gp_o[m].max()) - swb + 1
            fresh = (swb + wrows) > prev_tb
            gr_blk = np.full((C, n_s), -1, np.int64)
            par_blk = np.full((C, n_s), -1, np.int64)
            for c in range(C):
                mc = m & (gc_o == c)
                k = int(mc.sum())
                ii = np.zeros(n_s, np.int16)
                ii[:k] = (gp_o[mc] - swb).astype(np.int16)
                idxA_core[c].append(ii)
                gr_blk[c, :k] = gr_o[mc]
                par_blk[c, :k] = gq_o[mc]
            gr_cols.append(gr_blk)
            par_cols.append(par_blk)
            wins.append((swb, wrows, n_s, bool(fresh)))
        slot_gr = np.concatenate(gr_cols, axis=1)      # [C, A_pad]
        slot_par = np.concatenate(par_cols, axis=1)
        A_pad = slot_gr.shape[1]
        nch = A_pad // P
        idxA = np.stack([np.concatenate(idxA_core[c]) for c in range(C)])
        idxA_list.append(_wrap_idx(idxA))

        # chunk-freshness: chunk overlapping any fresh window's slot range
        fresh_chunk = np.zeros(nch, bool)
        off = 0
        for (swb, wrows, n_s, fresh) in wins:
            if fresh:
                fresh_chunk[off // P: _ceil(off + n_s, P)] = True
            off += n_s

        # pieces: union over cores of (chunk, dest window)
        nwin = _ceil(Lp, WW)
        piece_set = set()
        for c in range(C):
            g = slot_gr[c]
            for j in range(nch):
                sl = g[j * P:(j + 1) * P]
                sl = sl[sl >= 0]
                if sl.size:
                    for k in range(int(sl.min()) // WW, int(sl.max()) // WW + 1):
                        piece_set.add((j, k))
        # order: group by dest window; old chunks first, fresh last
        pieces = sorted(piece_set,
                        key=lambda jk: (jk[1], bool(fresh_chunk[jk[0]]), jk[0]))
        npieces = len(pieces)
        pcd = np.full((C, P, max(npieces, 1)), -30000.0, np.float32)
        for pi, (j, k) in enumerate(pieces):
            for c in range(C):
                sl = slot_gr[c, j * P:(j + 1) * P]
                v = np.where(sl >= 0, sl - k * WW, -30000)
                pcd[c, :, pi] = v
        par2 = np.zeros((C, P, 2 * nch), np.float32)
        for c in range(C):
            for j in range(nch):
                sl = slot_par[c, j * P:(j + 1) * P]
                par2[c, :, 2 * j] = (sl == 0)
                par2[c, :, 2 * j + 1] = (sl == 1)
        pcd_list.append(pcd)
        par2_list.append(par2)

        # per dest window: chunk list (msg matmul grouping)
        wchunks = []
        for k in range(nwin):
            wchunks.append([w2 for w2 in range(k * (WW // P),
                                               min(pl.nw2, (k + 1) * (WW // P)))])
        pl.wins, pl.A_pad, pl.nch, pl.npieces = wins, A_pad, nch, npieces
        pl.a0, pl.pc0, pl.pq0 = a0, pc0, pq0
        pl.pieces, pl.wchunks = pieces, wchunks
        a0 += A_pad // 16
        pc0 += max(npieces, 1)
        pq0 += 2 * nch
        plans.append(pl)
        prev_tb = tb

    CNT_TOT = cnt0
    AGP = ag0

    # ---------------- readout layout (two-hop over pairs) ----------------
    onodes = np.flatnonzero(nt == 1)
    og = bt[onodes]
    use_b = bl[onodes] >= 1
    use_f = (~use_b) & (fl[onodes] >= 1)
    rp = np.where(use_b, pairb[onodes],
                  np.where(use_f, pairf[onodes], _H0P + code[onodes]))
    rq = np.where(use_b, parb[onodes], np.where(use_f, parf[onodes], 0))
    kg_list = []
    for g in range(B):
        n_g = int((og == g).sum())
        kg_list.append(max(1, _ceil(_ceil(max(n_g, 1), C), P)))
    NRCH = int(np.sum(kg_list))
    c0s = np.concatenate([[0], np.cumsum(kg_list)])[:-1]
    roff = np.full((C, P, NRCH), -1, np.int64)
    rpar = np.zeros((C, P, NRCH), np.int64)
    rgid = np.full((C, P, NRCH), -1.0, np.float32)
    graph_chunks = []
    for g in range(B):
        m = og == g
        npos = rp[m]
        npar = rq[m]
        n_g = npos.size
        graph_chunks.append((int(c0s[g]), kg_list[g]))
        if n_g == 0:
            continue
        c = np.arange(n_g) % C
        sq = np.arange(n_g) // C
        j = sq // P
        p = sq % P
        roff[c, p, int(c0s[g]) + j] = npos
        rgid[c, p, int(c0s[g]) + j] = g
        rpar[c, p, int(c0s[g]) + j] = npar
    pad_m = roff < 0
    roff[pad_m] = _MIP
    rpar[pad_m] = 0

    # readout two-hop: phase A per window (final order filtered), perm idxC
    roff_lin = np.ascontiguousarray(roff.transpose(0, 2, 1)).reshape(C, NRCH * P)
    S_ro = NRCH * P
    sw = roff_lin // _W
    sws = sorted(set(sw.reshape(-1).tolist()))
    ro_wins = []
    posA = np.zeros((C, S_ro), np.int64)
    idxA_core = [[] for _ in range(C)]
    base = 0
    for s in sws:
        swb = s * _W
        cnts_c = [(sw[c] == s).sum() for c in range(C)]
        n_s = max(128, _ceil(max(cnts_c), 128) * 128)
        wrows = int(roff_lin[sw == s].max()) - swb + 1
        for c in range(C):
            mc = sw[c] == s
            k = int(mc.sum())
            ii = np.zeros(n_s, np.int16)
            ii[:k] = (roff_lin[c, mc] - swb).astype(np.int16)
            idxA_core[c].append(ii)
            posA[c, mc] = base + np.arange(k)
        ro_wins.append((swb, wrows, n_s))
        base += n_s
    ro_A = base
    idxA_list.append(_wrap_idx(
        np.stack([np.concatenate(idxA_core[c]) for c in range(C)])))
    ro_a0 = a0
    a0 += ro_A // 16
    idxC = posA.astype(np.int16)  # [C, S_ro] perm into staging
    idxC_w = _wrap_idx(idxC)

    # gating step for each readout window: emit after plan t (frontier grows)
    frontier = [_HDRP]
    for (sweep, l, Lp, tb) in lvl_meta:
        frontier.append(tb + (C * (Lp // 2) if Lp else 0))
    # frontier[i+1] = rows written after lvl_meta[i]'s AllGather
    ro_gate = []
    for (swb, wrows, n_s) in ro_wins:
        need = swb + wrows
        t = 0
        while frontier[t] < need:
            t += 1
        ro_gate.append(t - 1)  # plan index (into lvl_meta) or -1 for header

    par2ro = np.zeros((C, P, 2 * NRCH), np.float32)
    q2ro = np.zeros((C, P, 2 * NRCH), np.float32)
    for c in range(C):
        for j in range(NRCH):
            e = rpar[c, :, j] == 0
            par2ro[c, :, 2 * j] = e
            par2ro[c, :, 2 * j + 1] = ~e
            q2ro[c, :, 2 * j] = np.where(e, 0.0, BIG)
            q2ro[c, :, 2 * j + 1] = np.where(e, BIG, 0.0)

    # ---------------- weight-derived constants ----------------
    W_enc = np.asarray(W_enc, np.float32)
    b_enc = np.asarray(b_enc, np.float32)
    W_f = np.asarray(W_f, np.float32)
    b_f = np.asarray(b_f, np.float32)
    W_b = np.asarray(W_b, np.float32)
    b_b = np.asarray(b_b, np.float32)
    h0_tab = np.zeros((9, D), np.float32)
    for cc in range(9):
        h0_tab[cc] = (cc // 3) * W_enc[0] + (cc % 3) * W_enc[1] + b_enc
    tab = np.zeros((_HDRP, 2 * D), np.float32)
    tab[_H0P:_H0P + 9, 0:D] = h0_tab
    tab[_H0P:_H0P + 9, D:2 * D] = h0_tab
    tab[_MIP] = -BIG
    wtG = np.zeros((2 * D, 2 * D), np.float32)
    wtG[0:D, 0:D] = W_f
    wtG[D:2 * D, 0:D] = W_f
    wtG[0:D, D:2 * D] = W_b
    wtG[D:2 * D, D:2 * D] = W_b
    wtC = np.zeros((10, 2 * D), np.float32)
    wtC[0:9, 0:D] = h0_tab @ W_f
    wtC[9, 0:D] = b_f
    wtC[0:9, D:2 * D] = h0_tab @ W_b
    wtC[9, D:2 * D] = b_b

    iota512 = np.ascontiguousarray(
        np.tile(np.arange(512, dtype=np.float32), (P, 1)))
    iota16 = np.ascontiguousarray(
        np.tile(np.arange(16, dtype=np.float32), (P, 1)))

    f16 = np.float16
    idxA_all = np.concatenate(idxA_list, axis=2)
    pcd_all = (np.concatenate(pcd_list, axis=2) if pcd_list
               else np.zeros((C, P, 1), np.float32))
    par2_all = (np.concatenate(par2_list, axis=2) if par2_list
                else np.zeros((C, P, 2), np.float32))
    cnts_all = np.concatenate(cnts_list, axis=2)

    meta = dict(
        plans=plans, graph_chunks=graph_chunks, lvl_meta=lvl_meta,
        TP=TP, AGP=max(1, AGP), CNT_TOT=max(1, CNT_TOT), NRCH=NRCH,
        AW=idxA_all.shape[2], PCW=pcd_all.shape[2], PQW=par2_all.shape[2],
        CW=idxC_w.shape[2],
        ro=dict(wins=ro_wins, A_pad=ro_A, a0=ro_a0, gate=ro_gate),
    )
    arrays = dict(
        tab=tab.astype(f16), wtG=wtG.astype(f16), wtC=wtC.astype(f16),
        iota512=iota512.astype(f16), iota16=iota16.astype(f16),
        idxA=idxA_all, idxC=idxC_w,
        pcd=pcd_all.astype(f16), par2=par2_all.astype(f16),
        cnts=cnts_all.astype(f16), rgid=rgid.astype(f16),
        par2ro=par2ro.astype(f16), q2ro=q2ro.astype(f16),
    )
    return meta, arrays


# ---------------------------------------------------------------------------
# pure-numpy execution of the plan (host self-check / debugging)
# ---------------------------------------------------------------------------

def _simulate_plan(meta, arrays, return_T=False):
    f16 = np.float16
    TP = meta["TP"]
    T = np.zeros((TP, 2 * D), f16)
    T[0:_HDRP] = arrays["tab"]
    wtG = arrays["wtG"].astype(np.float32)
    wtC = arrays["wtC"].astype(np.float32)
    for pl in meta["plans"]:
        wcol = 0 if pl.sweep == "f" else D
        blocks = []
        for c in range(C):
            # gather phase-A G
            G = np.zeros((max(pl.A_pad, P), 2 * D), f16)
            off = 0
            a = pl.a0
            idxA = arrays["idxA"][c][0:16, :]
            for (swb, wrows, n_s, fresh) in pl.wins:
                cols = slice(a, a + n_s // 16)
                ii = idxA[:, cols].T.reshape(-1).astype(np.int64)
                G[off:off + n_s] = T[swb + ii]
                off += n_s
                a += n_s // 16
            # parity mask
            par2 = arrays["par2"][c][:, pl.pq0:pl.pq0 + 2 * pl.nch]
            Gv = G[:pl.A_pad].reshape(pl.nch, P, 2, D)
            for j in range(pl.nch):
                Gv[j, :, 0, :] *= par2[:, 2 * j][:, None].astype(f16)
                Gv[j, :, 1, :] *= par2[:, 2 * j + 1][:, None].astype(f16)
            # selection matmuls into agg [128, Lp] (fp32 psum)
            agg = np.zeros((2 * D, pl.Lp), np.float32)
            pcd = arrays["pcd"][c][:, pl.pc0:pl.pc0 + max(pl.npieces, 1)]
            for pi, (j, k) in enumerate(pl.pieces):
                width = min(WW, pl.Lp - k * WW)
                iota = np.arange(width, dtype=np.float32)
                S = (pcd[:, pi].astype(np.float32)[:, None] ==
                     iota[None, :]).astype(f16)
                Gc = G[j * P:(j + 1) * P].astype(np.float32)
                agg[:, k * WW:k * WW + width] += Gc.T @ S.astype(np.float32)
            lhsG = agg.astype(f16)
            cnt = arrays["cnts"][c][:, pl.cnt0:pl.cnt0 + pl.Lp]
            msg = (lhsG.astype(np.float32).T @ wtG[:, wcol:wcol + D]
                   + cnt.astype(np.float32).T @ wtC[:, wcol:wcol + D])
            blocks.append(msg.astype(f16))  # [Lp, D]
        # agin pair-rows + AllGather
        for c in range(C):
            msg = blocks[c]
            pair = np.zeros((pl.pr, 2 * D), f16)
            pair[:, 0:D] = msg[0:pl.half]
            pair[:, D:2 * D] = msg[pl.half:pl.Lp]
            T[pl.tbp + c * pl.pr: pl.tbp + (c + 1) * pl.pr] = pair

    # readout
    ro = meta["ro"]
    NRCH = meta["NRCH"]
    maxp = np.full((B, D), -np.inf, np.float32)
    sump = np.zeros((B, D), np.float32)
    for c in range(C):
        stg = np.zeros((ro["A_pad"], 2 * D), f16)
        off = 0
        a = ro["a0"]
        idxA = arrays["idxA"][c][0:16, :]
        for (swb, wrows, n_s) in ro["wins"]:
            cols = slice(a, a + n_s // 16)
            ii = idxA[:, cols].T.reshape(-1).astype(np.int64)
            stg[off:off + n_s] = T[swb + ii]
            off += n_s
            a += n_s // 16
        idxC = arrays["idxC"][c][0:16, :]
        perm = idxC.T.reshape(-1).astype(np.int64)  # [NRCH*P]
        R = stg[perm].reshape(NRCH, P, 2 * D).transpose(1, 0, 2)  # [P,NRCH,128]
        par2ro = arrays["par2ro"][c].astype(np.float32)  # [P, 2*NRCH]
        q2ro = arrays["q2ro"][c].astype(np.float32)
        Rf = R.astype(np.float32)
        Rm0 = np.empty_like(Rf)
        RmI = np.empty_like(Rf)
        for j in range(NRCH):
            for h in range(2):
                pa = par2ro[:, 2 * j + h][:, None]
                q = q2ro[:, 2 * j + h][:, None]
                Rm0[:, j, h * D:(h + 1) * D] = (
                    Rf[:, j, h * D:(h + 1) * D].astype(f16) * pa).astype(f16)
                RmI[:, j, h * D:(h + 1) * D] = (
                    Rf[:, j, h * D:(h + 1) * D].astype(f16) * pa - q).astype(f16)
        rgid = arrays["rgid"][c].astype(np.float32)  # [P, NRCH]
        psR = np.zeros((B, 2 * D), np.float32)
        for j in range(NRCH):
            S16 = (rgid[:, j][:, None] ==
                   np.arange(B, dtype=np.float32)[None, :])
            psR += S16.T.astype(np.float32) @ Rm0[:, j].astype(np.float32)
        sump += psR[:, 0:D] + psR[:, D:2 * D]
        for g, (c0, kg) in enumerate(meta["graph_chunks"]):
            mx = RmI[:, c0]
            for q_ in range(1, kg):
                mx = np.maximum(mx, RmI[:, c0 + q_])
            mm = mx.max(axis=0)
            maxp[g] = np.maximum(maxp[g], np.maximum(mm[0:D], mm[D:2 * D]))
    out = np.concatenate([maxp, sump], axis=1).astype(np.float32)
    return (out, T) if return_T else out


# ---------------------------------------------------------------------------
# Bass program
# ---------------------------------------------------------------------------

def _build(meta):
    import concourse.bass as bass
    import concourse.mybir as mybir
    from concourse import bacc, tile
    from concourse.masks import make_identity
    import os

    f32 = mybir.dt.float32
    f16 = mybir.dt.float16
    i16 = mybir.dt.int16
    AX = mybir.AxisListType
    OP = mybir.AluOpType
    ACT = mybir.ActivationFunctionType

    TP, AGP, CNT_TOT, NRCH = (meta["TP"], meta["AGP"], meta["CNT_TOT"],
                              meta["NRCH"])
    AW, PCW, PQW, CW = meta["AW"], meta["PCW"], meta["PQW"], meta["CW"]
    ro = meta["ro"]

    nc = bacc.Bacc(None, num_devices=C)
    tab_x = nc.dram_tensor("tab", [_HDRP, 2 * D], f16, kind="ExternalInput")
    wtG_x = nc.dram_tensor("wtG", [2 * D, 2 * D], f16, kind="ExternalInput")
    wtC_x = nc.dram_tensor("wtC", [10, 2 * D], f16, kind="ExternalInput")
    iota_x = nc.dram_tensor("iota", [P, 512], f16, kind="ExternalInput")
    io16_x = nc.dram_tensor("iota16", [P, 16], f16, kind="ExternalInput")
    idxA_x = nc.dram_tensor("idxA", [P, AW], i16, kind="ExternalInput")
    idxC_x = nc.dram_tensor("idxC", [P, CW], i16, kind="ExternalInput")
    pcd_x = nc.dram_tensor("pcd", [P, PCW], f16, kind="ExternalInput")
    par2_x = nc.dram_tensor("par2", [P, PQW], f16, kind="ExternalInput")
    cnts_x = nc.dram_tensor("cnts", [10, CNT_TOT], f16, kind="ExternalInput")
    rgid_x = nc.dram_tensor("rgid", [P, NRCH], f16, kind="ExternalInput")
    p2ro_x = nc.dram_tensor("par2ro", [P, 2 * NRCH], f16, kind="ExternalInput")
    q2ro_x = nc.dram_tensor("q2ro", [P, 2 * NRCH], f16, kind="ExternalInput")
    out_x = nc.dram_tensor("out", [B, 2 * D], f32, kind="ExternalOutput")

    T = nc.dram_tensor("T", [TP, 2 * D], f16, addr_space="Shared")
    dump_T = bool(os.environ.get("KERNEL_DUMP_T"))
    tdump_x = (nc.dram_tensor("tdump", [TP, 2 * D], f16,
                              kind="ExternalOutput") if dump_T else None)
    agin = nc.dram_tensor("agin", [AGP, 2 * D], f16)
    stg_t = nc.dram_tensor("stg", [max(ro["A_pad"], 128), 2 * D], f16)
    prin = nc.dram_tensor("prin", [2 * B, D], f32)
    prout = nc.dram_tensor("prout", [C * 2 * B, D], f32, addr_space="Shared")

    rg = [list(range(C))]
    max_lv = int(os.environ.get("KERNEL_MAX_LEVELS", "99"))
    skip_ro = bool(os.environ.get("KERNEL_SKIP_READOUT")) or max_lv < 99

    # map plan list indices to lvl_meta indices for readout gating
    plan_gate = {}  # lvl_meta index -> list of readout window ids
    for wi, t in enumerate(ro["gate"]):
        plan_gate.setdefault(t, []).append(wi)
    ro_w_off = []  # (a-col offset, slot offset) per readout window
    _a, _o = ro["a0"], 0
    for (swb, wrows, n_s) in ro["wins"]:
        ro_w_off.append((_a, _o))
        _a += n_s // 16
        _o += n_s

    with tile.TileContext(nc) as tc:
        with (
            tc.tile_pool(name="const", bufs=1) as constp,
            tc.tile_pool(name="g", bufs=2) as gpool,
            tc.tile_pool(name="s", bufs=3) as spool,
            tc.tile_pool(name="lhs", bufs=2) as lhsp,
            tc.tile_pool(name="msg", bufs=3) as msgp,
            tc.tile_pool(name="small", bufs=2) as smp,
            tc.tile_pool(name="psA", bufs=2, space="PSUM") as psA,
            tc.tile_pool(name="psB", bufs=2, space="PSUM") as psB,
            tc.tile_pool(name="psC", bufs=2, space="PSUM") as psC,
            tc.tile_pool(name="psR", bufs=1, space="PSUM") as psR,
        ):
            # ---------------- setup ----------------
            stg0 = constp.tile([_HDRP, 2 * D], f16)
            nc.sync.dma_start(out=stg0[:, :], in_=tab_x[:, :])
            nc.sync.dma_start(out=T[0:_HDRP, :], in_=stg0[:, :])
            wtg = constp.tile([2 * D, 2 * D], f16)
            nc.sync.dma_start(out=wtg[:, :], in_=wtG_x[:, :])
            wtc = constp.tile([10, 2 * D], f16)
            nc.sync.dma_start(out=wtc[:, :], in_=wtC_x[:, :])
            iot = constp.tile([P, 512], f16)
            nc.sync.dma_start(out=iot[:, :], in_=iota_x[:, :])
            io16 = constp.tile([P, 16], f16)
            nc.sync.dma_start(out=io16[:, :], in_=io16_x[:, :])
            idxAt = constp.tile([P, AW], i16)
            nc.sync.dma_start(out=idxAt[:, :], in_=idxA_x[:, :])
            idxCt = constp.tile([P, CW], i16)
            nc.sync.dma_start(out=idxCt[:, :], in_=idxC_x[:, :])
            pcdt = constp.tile([P, PCW], f16)
            nc.sync.dma_start(out=pcdt[:, :], in_=pcd_x[:, :])
            par2t = constp.tile([P, PQW], f16)
            nc.sync.dma_start(out=par2t[:, :], in_=par2_x[:, :])
            rgidt = constp.tile([P, NRCH], f16)
            nc.sync.dma_start(out=rgidt[:, :], in_=rgid_x[:, :])
            p2rot = constp.tile([P, 2 * NRCH], f16)
            nc.sync.dma_start(out=p2rot[:, :], in_=p2ro_x[:, :])
            q2rot = constp.tile([P, 2 * NRCH], f16)
            nc.sync.dma_start(out=q2rot[:, :], in_=q2ro_x[:, :])
            ident = constp.tile([P, P], f16)
            make_identity(nc, ident[:, :])
            roGA = constp.tile([P, (ro["A_pad"] // P) * 2 * D], f16)

            def emit_ro_windows(t):
                if skip_ro:
                    return
                for wi in plan_gate.get(t, []):
                    (swb, wrows, n_s) = ro["wins"][wi]
                    (acol, soff) = ro_w_off[wi]
                    j0 = soff // P
                    k = n_s // P
                    nc.gpsimd.dma_gather(
                        out_ap=roGA[:, j0 * 2 * D:(j0 + k) * 2 * D].rearrange(
                            "p (j f) -> p j f", f=2 * D),
                        in_ap=T[swb:swb + wrows, :],
                        idxs_ap=idxAt[:, acol:acol + n_s // 16],
                        num_idxs=n_s, num_idxs_reg=n_s, elem_size=2 * D,
                        single_packet=False)

            emit_ro_windows(-1)

            # ---------------- level sweeps ----------------
            for t, pl in enumerate(meta["plans"][:max_lv]):
                Lp, nch, nw2 = pl.Lp, pl.nch, pl.nw2
                nwin = _ceil(Lp, WW)
                wcol = 0 if pl.sweep == "f" else D
                cntt = lhsp.tile([10, Lp], f16, tag="cnt")
                nc.sync.dma_start(
                    out=cntt[:, :], in_=cnts_x[:, pl.cnt0:pl.cnt0 + Lp])
                lhsG = lhsp.tile([2 * D, Lp], f16, tag="lhsG")
                if nch > 0:
                    # S builds (no data deps - overlap with gathers)
                    stiles = {}
                    for k in range(nwin):
                        pieces_k = [pi for pi, (j, kk) in enumerate(pl.pieces)
                                    if kk == k]
                        if not pieces_k:
                            continue
                        # pieces of window k are contiguous in pcd order
                        pi0 = pieces_k[0]
                        npk = len(pieces_k)
                        width = min(WW, Lp - k * WW)
                        St = spool.tile([P, npk * width], f16, tag="S")
                        ia = iot[:, 0:width]
                        iota_rep = bass.AP(
                            ia.tensor, ia.offset,
                            [ia.ap[0], [0, npk], [1, width]])
                        pcb = pcdt[:, pl.pc0 + pi0:pl.pc0 + pi0 + npk]\
                            .to_broadcast([P, npk, width])
                        nc.vector.tensor_tensor(
                            out=St[:, :].rearrange("p (a b) -> p a b", a=npk),
                            in0=iota_rep, in1=pcb, op=OP.is_equal)
                        stiles[k] = (St, pi0, width)
                    # gathers (phase-A order), then parity mask per window run
                    G = gpool.tile([P, nch * 2 * D], f16, tag="G")
                    off = 0
                    a = pl.a0
                    for (swb, wrows, n_s, fresh) in pl.wins:
                        j0 = off // P
                        k = n_s // P
                        nc.gpsimd.dma_gather(
                            out_ap=G[:, j0 * 2 * D:(j0 + k) * 2 * D].rearrange(
                                "p (j f) -> p j f", f=2 * D),
                            in_ap=T[swb:swb + wrows, :],
                            idxs_ap=idxAt[:, a:a + n_s // 16],
                            num_idxs=n_s, num_idxs_reg=n_s, elem_size=2 * D,
                            single_packet=False)
                        # parity mask for this run's chunks (in place)
                        pq = par2t[:, pl.pq0 + 2 * j0:pl.pq0 + 2 * (j0 + k)]\
                            .to_broadcast([P, 2 * k, D])
                        gv = G[:, j0 * 2 * D:(j0 + k) * 2 * D].rearrange(
                            "p (a b) -> p a b", a=2 * k)
                        nc.vector.tensor_tensor(
                            out=gv, in0=gv, in1=pq, op=OP.mult)
                        off += n_s
                        a += n_s // 16
                    # selection matmuls per dest window, then lhs copy + msg
                    for k in range(nwin):
                        width = min(WW, Lp - k * WW)
                        if k not in stiles:
                            nc.vector.memset(
                                lhsG[:, k * WW:k * WW + width], 0.0)
                        else:
                            (St, pi0, width_) = stiles[k]
                            pieces_k = [(pi, j) for pi, (j, kk)
                                        in enumerate(pl.pieces) if kk == k]
                            ps = psA.tile([2 * D, width], f32, tag="psA")
                            for i, (pi, j) in enumerate(pieces_k):
                                nc.tensor.matmul(
                                    out=ps[:, :],
                                    lhsT=G[:, j * 2 * D:(j + 1) * 2 * D],
                                    rhs=St[:, (pi - pi0) * width:
                                           (pi - pi0 + 1) * width],
                                    start=(i == 0),
                                    stop=(i == len(pieces_k) - 1))
                            nc.scalar.activation(
                                out=lhsG[:, k * WW:k * WW + width],
                                in_=ps[:, :], func=ACT.Copy)
                        # msg matmuls for chunks in this window
                        for w2 in pl.wchunks[k]:
                            ps2 = psB.tile([P, D], f32, tag="psB")
                            nc.tensor.matmul(
                                out=ps2[:, :],
                                lhsT=lhsG[:, w2 * P:(w2 + 1) * P],
                                rhs=wtg[:, wcol:wcol + D],
                                start=True, stop=False)
                            nc.tensor.matmul(
                                out=ps2[:, :],
                                lhsT=cntt[:, w2 * P:(w2 + 1) * P],
                                rhs=wtc[:, wcol:wcol + D],
                                start=False, stop=True)
                            msgt = msgp.tile([P, D], f16, tag="msg")
                            nc.vector.tensor_copy(out=msgt[:, :], in_=ps2[:, :])
                            nhalf = nw2 // 2
                            if w2 < nhalf:
                                o_ap = agin[pl.ag0 + w2 * P:
                                            pl.ag0 + (w2 + 1) * P, 0:D]
                            else:
                                w2b = w2 - nhalf
                                o_ap = agin[pl.ag0 + w2b * P:
                                            pl.ag0 + (w2b + 1) * P, D:2 * D]
                            nc.sync.dma_start(out=o_ap, in_=msgt[:, :])
                else:
                    nc.vector.memset(lhsG[:, :], 0.0)
                    for w2 in range(nw2):
                        ps2 = psB.tile([P, D], f32, tag="psB")
                        nc.tensor.matmul(
                            out=ps2[:, :],
                            lhsT=lhsG[:, w2 * P:(w2 + 1) * P],
                            rhs=wtg[:, wcol:wcol + D],
                            start=True, stop=False)
                        nc.tensor.matmul(
                            out=ps2[:, :],
                            lhsT=cntt[:, w2 * P:(w2 + 1) * P],
                            rhs=wtc[:, wcol:wcol + D],
                            start=False, stop=True)
                        msgt = msgp.tile([P, D], f16, tag="msg")
                        nc.vector.tensor_copy(out=msgt[:, :], in_=ps2[:, :])
                        nhalf = nw2 // 2
                        if w2 < nhalf:
                            o_ap = agin[pl.ag0 + w2 * P:
                                        pl.ag0 + (w2 + 1) * P, 0:D]
                        else:
                            w2b = w2 - nhalf
                            o_ap = agin[pl.ag0 + w2b * P:
                                        pl.ag0 + (w2b + 1) * P, D:2 * D]
                        nc.sync.dma_start(out=o_ap, in_=msgt[:, :])
                nc.gpsimd.collective_compute(
                    "AllGather", OP.bypass, replica_groups=rg,
                    ins=[agin[pl.ag0:pl.ag0 + pl.pr, :]],
                    outs=[T[pl.tbp:pl.tbp + C * pl.pr, :]])
                emit_ro_windows(t)

            # ---------------- readout ----------------
            if skip_ro:
                zo = smp.tile([B, 2 * D], f32, tag="outt")
                nc.vector.memset(zo[:, :], 0.0)
                nc.sync.dma_start(out=out_x[:, :], in_=zo[:, :])
                if dump_T:
                    nc.sync.dma_start(out=tdump_x[:, :], in_=T[:, :])
            else:
                ro_stage = int(os.environ.get("KERNEL_RO_STAGE", "9"))

                def _zero_out():
                    zo = smp.tile([B, 2 * D], f32, tag="outt")
                    nc.vector.memset(zo[:, :], 0.0)
                    nc.sync.dma_start(out=out_x[:, :], in_=zo[:, :])
                    if dump_T:
                        nc.sync.dma_start(out=tdump_x[:, :], in_=T[:, :])

                # staging write + perm gather
                nc.sync.dma_start(
                    out=stg_t[0:ro["A_pad"], :].rearrange(
                        "(j p) f -> p j f", p=P),
                    in_=roGA[:, :].rearrange("p (j f) -> p j f", f=2 * D))
                R = constp.tile([P, NRCH * 2 * D], f16)
                nc.gpsimd.dma_gather(
                    out_ap=R[:, :].rearrange("p (j f) -> p j f", f=2 * D),
                    in_ap=stg_t[0:ro["A_pad"], :],
                    idxs_ap=idxCt[:, 0:(NRCH * P) // 16],
                    num_idxs=NRCH * P, num_idxs_reg=NRCH * P,
                    elem_size=2 * D, single_packet=False)
                if ro_stage <= 1:
                    _zero_out()
                else:
                    # mask wrong parity halves in place: R *= par2 (sum form)
                    rv = R[:, :].rearrange("p (a b) -> p a b", a=2 * NRCH)
                    p2b = p2rot[:, :].to_broadcast([P, 2 * NRCH, D])
                    q2b = q2rot[:, :].to_broadcast([P, 2 * NRCH, D])
                    nc.vector.tensor_tensor(out=rv, in0=rv, in1=p2b,
                                            op=OP.mult)
                    # sum pool: S16 one-hot matmuls
                    S16 = constp.tile([P, NRCH * 16], f16)
                    i16t = io16[:, 0:16]
                    i16_rep = bass.AP(
                        i16t.tensor, i16t.offset,
                        [i16t.ap[0], [0, NRCH], [1, 16]])
                    rgb = rgidt[:, :].to_broadcast([P, NRCH, 16])
                    nc.vector.tensor_tensor(
                        out=S16[:, :].rearrange("p (a b) -> p a b", a=NRCH),
                        in0=i16_rep, in1=rgb, op=OP.is_equal)
                    ps_sum = psR.tile([B, 2 * D], f32)
                    for j in range(NRCH):
                        nc.tensor.matmul(
                            out=ps_sum[:, :],
                            lhsT=S16[:, j * 16:(j + 1) * 16],
                            rhs=R[:, j * 2 * D:(j + 1) * 2 * D],
                            start=(j == 0), stop=(j == NRCH - 1))
                    sumsw = smp.tile([B, 2 * D], f32, tag="sumsw")
                    nc.vector.tensor_copy(out=sumsw[:, :], in_=ps_sum[:, :])
                    sums = smp.tile([B, D], f32, tag="sums")
                    nc.vector.tensor_tensor(
                        out=sums[:, :], in0=sumsw[:, 0:D],
                        in1=sumsw[:, D:2 * D], op=OP.add)
                if ro_stage == 2:
                    _zero_out()
                elif ro_stage > 2:
                    # max form in place: R -= q2 (pad/wrong halves -> -BIG)
                    nc.vector.tensor_tensor(out=rv, in0=rv, in1=q2b,
                                            op=OP.subtract)
                    # max pool: per-graph DVE max-tree + transpose + reduce
                    maxT = constp.tile([P, B], f16)
                    for g, (c0, kg) in enumerate(meta["graph_chunks"]):
                        if kg == 1:
                            mx_ap = R[:, c0 * 2 * D:(c0 + 1) * 2 * D]
                        else:
                            mx = smp.tile([P, 2 * D], f16, tag="mx")
                            nc.vector.tensor_tensor(
                                out=mx[:, :],
                                in0=R[:, c0 * 2 * D:(c0 + 1) * 2 * D],
                                in1=R[:, (c0 + 1) * 2 * D:(c0 + 2) * 2 * D],
                                op=OP.max)
                            for q_ in range(2, kg):
                                nc.vector.tensor_tensor(
                                    out=mx[:, :], in0=mx[:, :],
                                    in1=R[:, (c0 + q_) * 2 * D:
                                          (c0 + q_ + 1) * 2 * D],
                                    op=OP.max)
                            mx_ap = mx[:, :]
                        pst = psC.tile([2 * D, P], f16, tag="pst")
                        nc.tensor.transpose(
                            out=pst[:, :], in_=mx_ap, identity=ident[:, :])
                        nc.vector.reduce_max(
                            out=maxT[0:2 * D, g:g + 1], in_=pst[:, :],
                            axis=AX.X)
                    psmx = psC.tile([B, P], f16, tag="pst")
                    nc.tensor.transpose(
                        out=psmx[:, :], in_=maxT[:, :], identity=ident[:, :])
                    mxf = smp.tile([B, P], f16, tag="mxf")
                    nc.vector.tensor_copy(out=mxf[:, :], in_=psmx[:, :])
                    pr = smp.tile([B, 2 * D], f32, tag="pr")
                    nc.vector.tensor_tensor(
                        out=pr[:, 0:D], in0=mxf[:, 0:D], in1=mxf[:, D:2 * D],
                        op=OP.max)
                    nc.vector.tensor_copy(out=pr[:, D:2 * D], in_=sums[:, :])
                    if ro_stage == 3:
                        nc.sync.dma_start(out=out_x[:, :], in_=pr[:, :])
                        if dump_T:
                            nc.sync.dma_start(out=tdump_x[:, :], in_=T[:, :])
                    else:
                        nc.sync.dma_start(
                            out=prin[:, :].rearrange("(h g) f -> g h f", g=B),
                            in_=pr[:, :].rearrange("g (h f) -> g h f", h=2))
                        nc.gpsimd.collective_compute(
                            "AllGather", OP.bypass, replica_groups=rg,
                            ins=[prin[:, :]], outs=[prout[:, :]])
                        pr3 = prout[:, :].rearrange("(r gg) f -> gg r f", r=C)
                        mx8 = smp.tile([B, C * D], f32, tag="mx8")
                        sm8 = smp.tile([B, C * D], f32, tag="sm8")
                        nc.sync.dma_start(
                            out=mx8[:, :].rearrange("g (r f) -> g r f", f=D),
                            in_=pr3[0:B])
                        nc.sync.dma_start(
                            out=sm8[:, :].rearrange("g (r f) -> g r f", f=D),
                            in_=pr3[B:2 * B])
                        outt = smp.tile([B, 2 * D], f32, tag="outt")
                        t1m = smp.tile([B, 4 * D], f32, tag="t1m")
                        t2m = smp.tile([B, 2 * D], f32, tag="t2m")
                        nc.vector.tensor_tensor(
                            out=t1m[:, :], in0=mx8[:, 0:4 * D],
                            in1=mx8[:, 4 * D:8 * D], op=OP.max)
                        nc.vector.tensor_tensor(
                            out=t2m[:, :], in0=t1m[:, 0:2 * D],
                            in1=t1m[:, 2 * D:4 * D], op=OP.max)
                        nc.vector.tensor_tensor(
                            out=outt[:, 0:D], in0=t2m[:, 0:D],
                            in1=t2m[:, D:2 * D], op=OP.max)
                        t1s = smp.tile([B, 4 * D], f32, tag="t1s")
                        t2s = smp.tile([B, 2 * D], f32, tag="t2s")
                        nc.vector.tensor_tensor(
                            out=t1s[:, :], in0=sm8[:, 0:4 * D],
                            in1=sm8[:, 4 * D:8 * D], op=OP.add)
                        nc.vector.tensor_tensor(
                            out=t2s[:, :], in0=t1s[:, 0:2 * D],
                            in1=t1s[:, 2 * D:4 * D], op=OP.add)
                        nc.vector.tensor_tensor(
                            out=outt[:, D:2 * D], in0=t2s[:, 0:D],
                            in1=t2s[:, D:2 * D], op=OP.add)
                        nc.sync.dma_start(out=out_x[:, :], in_=outt[:, :])
                        if dump_T:
                            nc.sync.dma_start(out=tdump_x[:, :], in_=T[:, :])
    nc.compile()
    return nc


def _in_maps(meta, arrays):
    maps = []
    for c in range(C):
        maps.append(dict(
            tab=arrays["tab"],
            wtG=arrays["wtG"],
            wtC=arrays["wtC"],
            iota=arrays["iota512"],
            iota16=arrays["iota16"],
            idxA=np.ascontiguousarray(arrays["idxA"][c]),
            idxC=np.ascontiguousarray(arrays["idxC"][c]),
            pcd=np.ascontiguousarray(arrays["pcd"][c]),
            par2=np.ascontiguousarray(arrays["par2"][c]),
            cnts=np.ascontiguousarray(arrays["cnts"][c]),
            rgid=np.ascontiguousarray(arrays["rgid"][c]),
            par2ro=np.ascontiguousarray(arrays["par2ro"][c]),
            q2ro=np.ascontiguousarray(arrays["q2ro"][c]),
        ))
    return maps


_LAST_RESULTS = None  # stash for test harness (exec time, trace)


def kernel(**inputs):
    global _LAST_RESULTS
    import os
    meta, arrays = _preprocess(**inputs)
    nc = _build(meta)
    from concourse.bass_utils import run_bass_kernel_spmd
    res = run_bass_kernel_spmd(nc, _in_maps(meta, arrays),
                               core_ids=list(range(C)),
                               trace=bool(os.environ.get("KERNEL_TRACE")))
    _LAST_RESULTS = res
    return np.asarray(res.results[0]["out"])


# revision 17
# speedup vs baseline: 1.7397x; 1.3134x over previous
"""DAG-GNN level-sweep kernel for Trainium2 (8 NeuronCores, Bass/Tile), v3.

Single-hop gather design:
  - T table in DRAM stores fp16 *pair rows*: one 256B row = two node states
    (64 fp16 each). Gathers use int16 pair indices over 32767-pair windows
    (~3.8 level blocks per window), so each per-level gather is one hop in
    phase-A order (grouped by source window, sorted by dest rank) - no
    staging round-trip, no permutation gather.
  - A host-shipped 0/1 parity mask zeroes the unwanted half of each gathered
    pair (one bulk DVE multiply). The doubled feature rows fold back in the
    message matmul against [W; W]-stacked weights.
  - Segment-sum via PE matmuls of gather chunks against bulk-built one-hot
    selection matrices (per dest window of 512, fresh-source pieces last so
    old work overlaps the per-level AllGather).
  - Per-level fp16 AllGather of pair rows (half the bytes of f32).
  - Readout: two-hop (phase A overlapped into the backward sweep, small
    permutation gather at the end), parity-masked, sum via one-hot matmul,
    max via DVE max-tree + PE transpose.
"""

import sys

if "/opt/trn_rl_repo" not in sys.path:
    sys.path.insert(0, "/opt/trn_rl_repo")

import numpy as np

# structural constants of the nn.Module (match reference)
B = 16   # graphs per batch
LF = 12  # forward topological levels
LB = 12  # backward topological levels
D = 64   # hidden dim
C = 8    # NeuronCores
P = 128  # SBUF partitions
WW = 512  # dest window width for selection matmuls (one PSUM bank fp32)
BIG = 30000.0  # -BIG fills the readout max-padding row (fp16-safe)

# T header pair rows
_ZP = 0        # all-zero pair (gather padding target in window 0)
_H0P = 1       # 9 pair rows of initial-encoding table, indexed by 3*nt+nip
_MIP = 10      # [-BIG]*128 pair (readout max padding)
_HDRP = 12

_W = 32767  # dma_gather int16 source-window size (pair rows)


def _ceil(a, b):
    return -(-a // b)


class _Lvl:
    """Static per-level plan (identical across cores)."""

    __slots__ = (
        "sweep", "l", "Lp", "nw2", "half", "pr", "tbp", "cnt0", "ag0",
        "wins", "A_pad", "nch", "a0", "pc0", "pq0", "pieces", "npieces",
        "wchunks",
    )


def _wrap_idx(a):
    """[C, n] int16 -> [C, 128, n/16] (16-partition wrap, replicated x8)."""
    Csz, n = a.shape
    w = a.reshape(Csz, n // 16, 16).transpose(0, 2, 1)
    return np.ascontiguousarray(np.tile(w, (1, 8, 1)))


def _preprocess(node_type, num_inverted_predecessors, edge_index,
                forward_level, backward_level, batch,
                W_enc, b_enc, W_f, b_f, W_b, b_b):
    N = int(node_type.shape[0])
    nt = np.asarray(node_type).astype(np.int64)
    nip = np.asarray(num_inverted_predecessors).astype(np.int64)
    fl = np.asarray(forward_level).astype(np.int64)
    bl = np.asarray(backward_level).astype(np.int64)
    bt = np.asarray(batch).astype(np.int64)
    src = np.asarray(edge_index[0]).astype(np.int64)
    dst = np.asarray(edge_index[1]).astype(np.int64)
    code = nt * 3 + nip  # in [0, 9)

    # ---------------- node positions in pair-packed T ----------------
    pairf = np.full(N, -1, np.int64)
    parf = np.zeros(N, np.int64)
    pairb = np.full(N, -1, np.int64)
    parb = np.zeros(N, np.int64)
    coref = np.zeros(N, np.int64)
    rankf = np.zeros(N, np.int64)
    coreb = np.zeros(N, np.int64)
    rankb = np.zeros(N, np.int64)

    tbp = _HDRP
    lvl_meta = []  # (sweep, l, Lp, tbp)
    for sweep, lv, pair, par, core, rank, nl in (
        ("f", fl, pairf, parf, coref, rankf, LF),
        ("b", bl, pairb, parb, coreb, rankb, LB),
    ):
        for l in range(1, nl):
            idx = np.flatnonzero(lv == l)
            n_l = idx.size
            if n_l == 0:
                lvl_meta.append((sweep, l, 0, tbp))
                continue
            Lp = _ceil(_ceil(n_l, C), 256) * 256
            half = Lp // 2
            c = np.arange(n_l) % C
            r = np.arange(n_l) // C
            pair[idx] = tbp + c * half + (r % half)
            par[idx] = r // half
            core[idx] = c
            rank[idx] = r
            lvl_meta.append((sweep, l, Lp, tbp))
            tbp += C * half
    TP = tbp  # total T pair rows

    # ---------------- per-level plans ----------------
    plans = []
    idxA_list = []    # per-window wrapped idx blocks [C,128,n/16]
    pcd_list = []     # [C, P, npieces] f32 (cast f16 at ship)
    par2_list = []    # [C, P, 2*nch]
    cnts_list = []    # [C, 10, Lp]
    a0 = pc0 = pq0 = 0
    cnt0 = 0
    ag0 = 0
    fl_dst = fl[dst]
    bl_src = bl[src]
    prev_tb = _HDRP  # base pair of the block written one step earlier
    for (sweep, l, Lp, tb) in lvl_meta:
        if Lp == 0:
            prev_tb = tb
            continue
        pl = _Lvl()
        pl.sweep, pl.l, pl.Lp, pl.tbp = sweep, l, Lp, tb
        pl.nw2 = Lp // P
        pl.half = Lp // 2
        pl.pr = pl.half
        pl.cnt0 = cnt0
        pl.ag0 = ag0
        if sweep == "f":
            em = fl_dst == l
            un = dst[em]
            dn = src[em]
            ucore = coref[un]
            urank = rankf[un]
            gat = (fl[dn] >= 1) & (fl[dn] < l)
            gpair_all = pairf[dn]
            gpar_all = parf[dn]
        else:
            em = bl_src == l
            un = src[em]
            dn = dst[em]
            ucore = coreb[un]
            urank = rankb[un]
            upd_b = (bl[dn] >= 1) & (bl[dn] < l)
            upd_f = (~upd_b) & (fl[dn] >= 1)
            gat = upd_b | upd_f
            gpair_all = np.where(upd_b, pairb[dn], pairf[dn])
            gpar_all = np.where(upd_b, parb[dn], parf[dn])

        # counts: initial-valued sources by code, plus total indegree (bias)
        cnt = np.zeros((C, Lp, 10), np.float32)
        i0 = ~gat
        np.add.at(cnt, (ucore[i0], urank[i0], code[dn[i0]]), 1.0)
        np.add.at(cnt, (ucore, urank, 9), 1.0)
        cnts_list.append(np.ascontiguousarray(cnt.transpose(0, 2, 1)))
        cnt0 += Lp
        ag0 += pl.pr

        gc = ucore[gat]
        gr = urank[gat]
        gp = gpair_all[gat]
        gq = gpar_all[gat]
        if gp.size == 0:
            pl.wins, pl.A_pad, pl.nch, pl.npieces = [], 0, 0, 0
            pl.a0, pl.pc0, pl.pq0 = a0, pc0, pq0
            pl.pieces, pl.wchunks = [], []
            plans.append(pl)
            prev_tb = tb
            continue

        # windows (fixed 32767-pair grid), per-core slots sorted by dest rank
        sw = gp // _W
        sws = sorted(set(sw.tolist()))
        wins = []          # (swb, wrows, n_slots, fresh)
        idx16 = []         # per-core concatenated int16 idx
        slot_gr = np.full((C, 0), -1, np.int64)
        gr_cols = []
        par_cols = []
        order = np.lexsort((gr, sw, gc))
        gc_o, gr_o, gp_o, gq_o, sw_o = (
            gc[order], gr[order], gp[order], gq[order], sw[order])
        idxA_core = [[] for _ in range(C)]
        for s in sws:
            swb = s * _W
            m = sw_o == s
            cnts_c = np.bincount(gc_o[m], minlength=C)
            n_s = max(128, _ceil(int(cnts_c.max()), 128) * 128)
            wrows = int(gp_o[m].max()) - swb + 1
            fresh = (swb + wrows) > prev_tb
            gr_blk = np.full((C, n_s), -1, np.int64)
            par_blk = np.full((C, n_s), -1, np.int64)
            for c in range(C):
                mc = m & (gc_o == c)
                k = int(mc.sum())
                ii = np.zeros(n_s, np.int16)
                ii[:k] = (gp_o[mc] - swb).astype(np.int16)
                idxA_core[c].append(ii)
                gr_blk[c, :k] = gr_o[mc]
                par_blk[c, :k] = gq_o[mc]
            gr_cols.append(gr_blk)
            par_cols.append(par_blk)
            wins.append((swb, wrows, n_s, bool(fresh)))
        slot_gr = np.concatenate(gr_cols, axis=1)      # [C, A_pad]
        slot_par = np.concatenate(par_cols, axis=1)
        A_pad = slot_gr.shape[1]
        nch = A_pad // P
        idxA = np.stack([np.concatenate(idxA_core[c]) for c in range(C)])
        idxA_list.append(_wrap_idx(idxA))

        # chunk-freshness: chunk overlapping any fresh window's slot range
        fresh_chunk = np.zeros(nch, bool)
        off = 0
        for (swb, wrows, n_s, fresh) in wins:
            if fresh:
                fresh_chunk[off // P: _ceil(off + n_s, P)] = True
            off += n_s

        # pieces: union over cores of (chunk, dest window)
        nwin = _ceil(Lp, WW)
        piece_set = set()
        for c in range(C):
            g = slot_gr[c]
            for j in range(nch):
                sl = g[j * P:(j + 1) * P]
                sl = sl[sl >= 0]
                if sl.size:
                    for k in range(int(sl.min()) // WW, int(sl.max()) // WW + 1):
                        piece_set.add((j, k))
        # order: group by dest window; old chunks first, fresh last
        pieces = sorted(piece_set,
                        key=lambda jk: (jk[1], bool(fresh_chunk[jk[0]]), jk[0]))
        npieces = len(pieces)
        pcd = np.full((C, P, max(npieces, 1)), -30000.0, np.float32)
        for pi, (j, k) in enumerate(pieces):
            for c in range(C):
                sl = slot_gr[c, j * P:(j + 1) * P]
                v = np.where(sl >= 0, sl - k * WW, -30000)
                pcd[c, :, pi] = v
        par2 = np.zeros((C, P, 2 * nch), np.float32)
        for c in range(C):
            for j in range(nch):
                sl = slot_par[c, j * P:(j + 1) * P]
                par2[c, :, 2 * j] = (sl == 0)
                par2[c, :, 2 * j + 1] = (sl == 1)
        pcd_list.append(pcd)
        par2_list.append(par2)

        # per dest window: chunk list (msg matmul grouping)
        wchunks = []
        for k in range(nwin):
            wchunks.append([w2 for w2 in range(k * (WW // P),
                                               min(pl.nw2, (k + 1) * (WW // P)))])
        pl.wins, pl.A_pad, pl.nch, pl.npieces = wins, A_pad, nch, npieces
        pl.a0, pl.pc0, pl.pq0 = a0, pc0, pq0
        pl.pieces, pl.wchunks = pieces, wchunks
        a0 += A_pad // 16
        pc0 += max(npieces, 1)
        pq0 += 2 * nch
        plans.append(pl)
        prev_tb = tb

    CNT_TOT = cnt0
    AGP = ag0

    # ---------------- readout layout (two-hop over pairs) ----------------
    onodes = np.flatnonzero(nt == 1)
    og = bt[onodes]
    use_b = bl[onodes] >= 1
    use_f = (~use_b) & (fl[onodes] >= 1)
    rp = np.where(use_b, pairb[onodes],
                  np.where(use_f, pairf[onodes], _H0P + code[onodes]))
    rq = np.where(use_b, parb[onodes], np.where(use_f, parf[onodes], 0))
    kg_list = []
    for g in range(B):
        n_g = int((og == g).sum())
        kg_list.append(max(1, _ceil(_ceil(max(n_g, 1), C), P)))
    NRCH = int(np.sum(kg_list))
    c0s = np.concatenate([[0], np.cumsum(kg_list)])[:-1]
    roff = np.full((C, P, NRCH), -1, np.int64)
    rpar = np.zeros((C, P, NRCH), np.int64)
    rgid = np.full((C, P, NRCH), -1.0, np.float32)
    graph_chunks = []
    for g in range(B):
        m = og == g
        npos = rp[m]
        npar = rq[m]
        n_g = npos.size
        graph_chunks.append((int(c0s[g]), kg_list[g]))
        if n_g == 0:
            continue
        c = np.arange(n_g) % C
        sq = np.arange(n_g) // C
        j = sq // P
        p = sq % P
        roff[c, p, int(c0s[g]) + j] = npos
        rgid[c, p, int(c0s[g]) + j] = g
        rpar[c, p, int(c0s[g]) + j] = npar
    pad_m = roff < 0
    roff[pad_m] = _MIP
    rpar[pad_m] = 0

    # readout two-hop: phase A per window (final order filtered), perm idxC
    roff_lin = np.ascontiguousarray(roff.transpose(0, 2, 1)).reshape(C, NRCH * P)
    S_ro = NRCH * P
    sw = roff_lin // _W
    sws = sorted(set(sw.reshape(-1).tolist()))
    ro_wins = []
    posA = np.zeros((C, S_ro), np.int64)
    idxA_core = [[] for _ in range(C)]
    base = 0
    for s in sws:
        swb = s * _W
        cnts_c = [(sw[c] == s).sum() for c in range(C)]
        n_s = max(128, _ceil(max(cnts_c), 128) * 128)
        wrows = int(roff_lin[sw == s].max()) - swb + 1
        for c in range(C):
            mc = sw[c] == s
            k = int(mc.sum())
            ii = np.zeros(n_s, np.int16)
            ii[:k] = (roff_lin[c, mc] - swb).astype(np.int16)
            idxA_core[c].append(ii)
            posA[c, mc] = base + np.arange(k)
        ro_wins.append((swb, wrows, n_s))
        base += n_s
    ro_A = base
    idxA_list.append(_wrap_idx(
        np.stack([np.concatenate(idxA_core[c]) for c in range(C)])))
    ro_a0 = a0
    a0 += ro_A // 16
    idxC = posA.astype(np.int16)  # [C, S_ro] perm into staging
    idxC_w = _wrap_idx(idxC)

    # gating step for each readout window: emit after plan t (frontier grows)
    frontier = [_HDRP]
    for (sweep, l, Lp, tb) in lvl_meta:
        frontier.append(tb + (C * (Lp // 2) if Lp else 0))
    # frontier[i+1] = rows written after lvl_meta[i]'s AllGather
    ro_gate = []
    for (swb, wrows, n_s) in ro_wins:
        need = swb + wrows
        t = 0
        while frontier[t] < need:
            t += 1
        ro_gate.append(t - 1)  # plan index (into lvl_meta) or -1 for header

    par2ro = np.zeros((C, P, 2 * NRCH), np.float32)
    q2ro = np.zeros((C, P, 2 * NRCH), np.float32)
    for c in range(C):
        for j in range(NRCH):
            e = rpar[c, :, j] == 0
            par2ro[c, :, 2 * j] = e
            par2ro[c, :, 2 * j + 1] = ~e
            q2ro[c, :, 2 * j] = np.where(e, 0.0, BIG)
            q2ro[c, :, 2 * j + 1] = np.where(e, BIG, 0.0)

    # ---------------- weight-derived constants ----------------
    W_enc = np.asarray(W_enc, np.float32)
    b_enc = np.asarray(b_enc, np.float32)
    W_f = np.asarray(W_f, np.float32)
    b_f = np.asarray(b_f, np.float32)
    W_b = np.asarray(W_b, np.float32)
    b_b = np.asarray(b_b, np.float32)
    h0_tab = np.zeros((9, D), np.float32)
    for cc in range(9):
        h0_tab[cc] = (cc // 3) * W_enc[0] + (cc % 3) * W_enc[1] + b_enc
    tab = np.zeros((_HDRP, 2 * D), np.float32)
    tab[_H0P:_H0P + 9, 0:D] = h0_tab
    tab[_H0P:_H0P + 9, D:2 * D] = h0_tab
    tab[_MIP] = -BIG
    wtG = np.zeros((2 * D, 2 * D), np.float32)
    wtG[0:D, 0:D] = W_f
    wtG[D:2 * D, 0:D] = W_f
    wtG[0:D, D:2 * D] = W_b
    wtG[D:2 * D, D:2 * D] = W_b
    wtC = np.zeros((10, 2 * D), np.float32)
    wtC[0:9, 0:D] = h0_tab @ W_f
    wtC[9, 0:D] = b_f
    wtC[0:9, D:2 * D] = h0_tab @ W_b
    wtC[9, D:2 * D] = b_b

    iota512 = np.ascontiguousarray(
        np.tile(np.arange(512, dtype=np.float32), (P, 1)))
    iota16 = np.ascontiguousarray(
        np.tile(np.arange(16, dtype=np.float32), (P, 1)))

    f16 = np.float16
    idxA_all = np.concatenate(idxA_list, axis=2)
    pcd_all = (np.concatenate(pcd_list, axis=2) if pcd_list
               else np.zeros((C, P, 1), np.float32))
    par2_all = (np.concatenate(par2_list, axis=2) if par2_list
                else np.zeros((C, P, 2), np.float32))
    cnts_all = np.concatenate(cnts_list, axis=2)

    meta = dict(
        plans=plans, graph_chunks=graph_chunks, lvl_meta=lvl_meta,
        TP=TP, AGP=max(1, AGP), CNT_TOT=max(1, CNT_TOT), NRCH=NRCH,
        AW=idxA_all.shape[2], PCW=pcd_all.shape[2], PQW=par2_all.shape[2],
        CW=idxC_w.shape[2],
        ro=dict(wins=ro_wins, A_pad=ro_A, a0=ro_a0, gate=ro_gate),
    )
    arrays = dict(
        tab=tab.astype(f16), wtG=wtG.astype(f16), wtC=wtC.astype(f16),
        iota512=iota512.astype(f16), iota16=iota16.astype(f16),
        idxA=idxA_all, idxC=idxC_w,
        pcd=pcd_all.astype(f16), par2=par2_all.astype(f16),
        cnts=cnts_all.astype(f16), rgid=rgid.astype(f16),
        par2ro=par2ro.astype(f16), q2ro=q2ro.astype(f16),
    )
    return meta, arrays


# ---------------------------------------------------------------------------
# pure-numpy execution of the plan (host self-check / debugging)
# ---------------------------------------------------------------------------

def _simulate_plan(meta, arrays, return_T=False):
    f16 = np.float16
    TP = meta["TP"]
    T = np.zeros((TP, 2 * D), f16)
    T[0:_HDRP] = arrays["tab"]
    wtG = arrays["wtG"].astype(np.float32)
    wtC = arrays["wtC"].astype(np.float32)
    for pl in meta["plans"]:
        wcol = 0 if pl.sweep == "f" else D
        blocks = []
        for c in range(C):
            # gather phase-A G
            G = np.zeros((max(pl.A_pad, P), 2 * D), f16)
            off = 0
            a = pl.a0
            idxA = arrays["idxA"][c][0:16, :]
            for (swb, wrows, n_s, fresh) in pl.wins:
                cols = slice(a, a + n_s // 16)
                ii = idxA[:, cols].T.reshape(-1).astype(np.int64)
                G[off:off + n_s] = T[swb + ii]
                off += n_s
                a += n_s // 16
            # parity mask
            par2 = arrays["par2"][c][:, pl.pq0:pl.pq0 + 2 * pl.nch]
            Gv = G[:pl.A_pad].reshape(pl.nch, P, 2, D)
            for j in range(pl.nch):
                Gv[j, :, 0, :] *= par2[:, 2 * j][:, None].astype(f16)
                Gv[j, :, 1, :] *= par2[:, 2 * j + 1][:, None].astype(f16)
            # selection matmuls into agg [128, Lp] (fp32 psum)
            agg = np.zeros((2 * D, pl.Lp), np.float32)
            pcd = arrays["pcd"][c][:, pl.pc0:pl.pc0 + max(pl.npieces, 1)]
            for pi, (j, k) in enumerate(pl.pieces):
                width = min(WW, pl.Lp - k * WW)
                iota = np.arange(width, dtype=np.float32)
                S = (pcd[:, pi].astype(np.float32)[:, None] ==
                     iota[None, :]).astype(f16)
                Gc = G[j * P:(j + 1) * P].astype(np.float32)
                agg[:, k * WW:k * WW + width] += Gc.T @ S.astype(np.float32)
            lhsG = agg.astype(f16)
            cnt = arrays["cnts"][c][:, pl.cnt0:pl.cnt0 + pl.Lp]
            msg = (lhsG.astype(np.float32).T @ wtG[:, wcol:wcol + D]
                   + cnt.astype(np.float32).T @ wtC[:, wcol:wcol + D])
            blocks.append(msg.astype(f16))  # [Lp, D]
        # agin pair-rows + AllGather
        for c in range(C):
            msg = blocks[c]
            pair = np.zeros((pl.pr, 2 * D), f16)
            pair[:, 0:D] = msg[0:pl.half]
            pair[:, D:2 * D] = msg[pl.half:pl.Lp]
            T[pl.tbp + c * pl.pr: pl.tbp + (c + 1) * pl.pr] = pair

    # readout
    ro = meta["ro"]
    NRCH = meta["NRCH"]
    maxp = np.full((B, D), -np.inf, np.float32)
    sump = np.zeros((B, D), np.float32)
    for c in range(C):
        stg = np.zeros((ro["A_pad"], 2 * D), f16)
        off = 0
        a = ro["a0"]
        idxA = arrays["idxA"][c][0:16, :]
        for (swb, wrows, n_s) in ro["wins"]:
            cols = slice(a, a + n_s // 16)
            ii = idxA[:, cols].T.reshape(-1).astype(np.int64)
            stg[off:off + n_s] = T[swb + ii]
            off += n_s
            a += n_s // 16
        idxC = arrays["idxC"][c][0:16, :]
        perm = idxC.T.reshape(-1).astype(np.int64)  # [NRCH*P]
        R = stg[perm].reshape(NRCH, P, 2 * D).transpose(1, 0, 2)  # [P,NRCH,128]
        par2ro = arrays["par2ro"][c].astype(np.float32)  # [P, 2*NRCH]
        q2ro = arrays["q2ro"][c].astype(np.float32)
        Rf = R.astype(np.float32)
        Rm0 = np.empty_like(Rf)
        RmI = np.empty_like(Rf)
        for j in range(NRCH):
            for h in range(2):
                pa = par2ro[:, 2 * j + h][:, None]
                q = q2ro[:, 2 * j + h][:, None]
                Rm0[:, j, h * D:(h + 1) * D] = (
                    Rf[:, j, h * D:(h + 1) * D].astype(f16) * pa).astype(f16)
                RmI[:, j, h * D:(h + 1) * D] = (
                    Rf[:, j, h * D:(h + 1) * D].astype(f16) * pa - q).astype(f16)
        rgid = arrays["rgid"][c].astype(np.float32)  # [P, NRCH]
        psR = np.zeros((B, 2 * D), np.float32)
        for j in range(NRCH):
            S16 = (rgid[:, j][:, None] ==
                   np.arange(B, dtype=np.float32)[None, :])
            psR += S16.T.astype(np.float32) @ Rm0[:, j].astype(np.float32)
        sump += psR[:, 0:D] + psR[:, D:2 * D]
        for g, (c0, kg) in enumerate(meta["graph_chunks"]):
            mx = RmI[:, c0]
            for q_ in range(1, kg):
                mx = np.maximum(mx, RmI[:, c0 + q_])
            mm = mx.max(axis=0)
            maxp[g] = np.maximum(maxp[g], np.maximum(mm[0:D], mm[D:2 * D]))
    out = np.concatenate([maxp, sump], axis=1).astype(np.float32)
    return (out, T) if return_T else out


# ---------------------------------------------------------------------------
# Bass program
# ---------------------------------------------------------------------------

def _build(meta):
    import concourse.bass as bass
    import concourse.mybir as mybir
    from concourse import bacc, tile
    from concourse.masks import make_identity
    import os

    f32 = mybir.dt.float32
    f16 = mybir.dt.float16
    i16 = mybir.dt.int16
    AX = mybir.AxisListType
    OP = mybir.AluOpType
    ACT = mybir.ActivationFunctionType

    TP, AGP, CNT_TOT, NRCH = (meta["TP"], meta["AGP"], meta["CNT_TOT"],
                              meta["NRCH"])
    AW, PCW, PQW, CW = meta["AW"], meta["PCW"], meta["PQW"], meta["CW"]
    ro = meta["ro"]

    nc = bacc.Bacc(None, num_devices=C, num_swdge_queues=4,
                   dynamic_dma_scratch_size=32768)
    _qn = [0]

    def _next_q():
        q = _qn[0] % 4
        _qn[0] += 1
        return q
    tab_x = nc.dram_tensor("tab", [_HDRP, 2 * D], f16, kind="ExternalInput")
    wtG_x = nc.dram_tensor("wtG", [2 * D, 2 * D], f16, kind="ExternalInput")
    wtC_x = nc.dram_tensor("wtC", [10, 2 * D], f16, kind="ExternalInput")
    iota_x = nc.dram_tensor("iota", [P, 512], f16, kind="ExternalInput")
    io16_x = nc.dram_tensor("iota16", [P, 16], f16, kind="ExternalInput")
    idxA_x = nc.dram_tensor("idxA", [P, AW], i16, kind="ExternalInput")
    idxC_x = nc.dram_tensor("idxC", [P, CW], i16, kind="ExternalInput")
    pcd_x = nc.dram_tensor("pcd", [P, PCW], f16, kind="ExternalInput")
    par2_x = nc.dram_tensor("par2", [P, PQW], f16, kind="ExternalInput")
    cnts_x = nc.dram_tensor("cnts", [10, CNT_TOT], f16, kind="ExternalInput")
    rgid_x = nc.dram_tensor("rgid", [P, NRCH], f16, kind="ExternalInput")
    p2ro_x = nc.dram_tensor("par2ro", [P, 2 * NRCH], f16, kind="ExternalInput")
    q2ro_x = nc.dram_tensor("q2ro", [P, 2 * NRCH], f16, kind="ExternalInput")
    out_x = nc.dram_tensor("out", [B, 2 * D], f32, kind="ExternalOutput")

    T = nc.dram_tensor("T", [TP, 2 * D], f16, addr_space="Shared")
    dump_T = bool(os.environ.get("KERNEL_DUMP_T"))
    tdump_x = (nc.dram_tensor("tdump", [TP, 2 * D], f16,
                              kind="ExternalOutput") if dump_T else None)
    agin = nc.dram_tensor("agin", [AGP, 2 * D], f16)
    stg_t = nc.dram_tensor("stg", [max(ro["A_pad"], 128), 2 * D], f16)
    prin = nc.dram_tensor("prin", [2 * B, D], f32)
    prout = nc.dram_tensor("prout", [C * 2 * B, D], f32, addr_space="Shared")

    rg = [list(range(C))]
    max_lv = int(os.environ.get("KERNEL_MAX_LEVELS", "99"))
    skip_ro = bool(os.environ.get("KERNEL_SKIP_READOUT")) or max_lv < 99

    # map plan list indices to lvl_meta indices for readout gating
    plan_gate = {}  # lvl_meta index -> list of readout window ids
    for wi, t in enumerate(ro["gate"]):
        plan_gate.setdefault(t, []).append(wi)
    ro_w_off = []  # (a-col offset, slot offset) per readout window
    _a, _o = ro["a0"], 0
    for (swb, wrows, n_s) in ro["wins"]:
        ro_w_off.append((_a, _o))
        _a += n_s // 16
        _o += n_s

    with tile.TileContext(nc) as tc:
        with (
            tc.tile_pool(name="const", bufs=1) as constp,
            tc.tile_pool(name="g", bufs=2) as gpool,
            tc.tile_pool(name="s", bufs=3) as spool,
            tc.tile_pool(name="lhs", bufs=2) as lhsp,
            tc.tile_pool(name="msg", bufs=3) as msgp,
            tc.tile_pool(name="small", bufs=2) as smp,
            tc.tile_pool(name="psA", bufs=2, space="PSUM") as psA,
            tc.tile_pool(name="psB", bufs=2, space="PSUM") as psB,
            tc.tile_pool(name="psC", bufs=2, space="PSUM") as psC,
            tc.tile_pool(name="psR", bufs=1, space="PSUM") as psR,
        ):
            # ---------------- setup ----------------
            stg0 = constp.tile([_HDRP, 2 * D], f16)
            nc.sync.dma_start(out=stg0[:, :], in_=tab_x[:, :])
            nc.sync.dma_start(out=T[0:_HDRP, :], in_=stg0[:, :])
            wtg = constp.tile([2 * D, 2 * D], f16)
            nc.sync.dma_start(out=wtg[:, :], in_=wtG_x[:, :])
            wtc = constp.tile([10, 2 * D], f16)
            nc.sync.dma_start(out=wtc[:, :], in_=wtC_x[:, :])
            iot = constp.tile([P, 512], f16)
            nc.sync.dma_start(out=iot[:, :], in_=iota_x[:, :])
            io16 = constp.tile([P, 16], f16)
            nc.sync.dma_start(out=io16[:, :], in_=io16_x[:, :])
            idxAt = constp.tile([P, AW], i16)
            nc.sync.dma_start(out=idxAt[:, :], in_=idxA_x[:, :])
            idxCt = constp.tile([P, CW], i16)
            nc.sync.dma_start(out=idxCt[:, :], in_=idxC_x[:, :])
            pcdt = constp.tile([P, PCW], f16)
            nc.sync.dma_start(out=pcdt[:, :], in_=pcd_x[:, :])
            par2t = constp.tile([P, PQW], f16)
            nc.sync.dma_start(out=par2t[:, :], in_=par2_x[:, :])
            rgidt = constp.tile([P, NRCH], f16)
            nc.sync.dma_start(out=rgidt[:, :], in_=rgid_x[:, :])
            p2rot = constp.tile([P, 2 * NRCH], f16)
            nc.sync.dma_start(out=p2rot[:, :], in_=p2ro_x[:, :])
            q2rot = constp.tile([P, 2 * NRCH], f16)
            nc.sync.dma_start(out=q2rot[:, :], in_=q2ro_x[:, :])
            ident = constp.tile([P, P], f16)
            make_identity(nc, ident[:, :])
            roGA = constp.tile([P, (ro["A_pad"] // P) * 2 * D], f16)

            def emit_ro_windows(t):
                if skip_ro:
                    return
                for wi in plan_gate.get(t, []):
                    (swb, wrows, n_s) = ro["wins"][wi]
                    (acol, soff) = ro_w_off[wi]
                    j0 = soff // P
                    k = n_s // P
                    nc.gpsimd.dma_gather(
                        out_ap=roGA[:, j0 * 2 * D:(j0 + k) * 2 * D].rearrange(
                            "p (j f) -> p j f", f=2 * D),
                        in_ap=T[swb:swb + wrows, :],
                        idxs_ap=idxAt[:, acol:acol + n_s // 16],
                        num_idxs=n_s, num_idxs_reg=n_s, elem_size=2 * D,
                        single_packet=False, queue_num=_next_q())

            emit_ro_windows(-1)

            # ---------------- level sweeps ----------------
            for t, pl in enumerate(meta["plans"][:max_lv]):
                Lp, nch, nw2 = pl.Lp, pl.nch, pl.nw2
                nwin = _ceil(Lp, WW)
                wcol = 0 if pl.sweep == "f" else D
                cntt = lhsp.tile([10, Lp], f16, tag="cnt")
                nc.sync.dma_start(
                    out=cntt[:, :], in_=cnts_x[:, pl.cnt0:pl.cnt0 + Lp])
                lhsG = lhsp.tile([2 * D, Lp], f16, tag="lhsG")
                if nch > 0:
                    # S builds (no data deps - overlap with gathers)
                    stiles = {}
                    for k in range(nwin):
                        pieces_k = [pi for pi, (j, kk) in enumerate(pl.pieces)
                                    if kk == k]
                        if not pieces_k:
                            continue
                        # pieces of window k are contiguous in pcd order
                        pi0 = pieces_k[0]
                        npk = len(pieces_k)
                        width = min(WW, Lp - k * WW)
                        St = spool.tile([P, npk * width], f16, tag="S")
                        ia = iot[:, 0:width]
                        iota_rep = bass.AP(
                            ia.tensor, ia.offset,
                            [ia.ap[0], [0, npk], [1, width]])
                        pcb = pcdt[:, pl.pc0 + pi0:pl.pc0 + pi0 + npk]\
                            .to_broadcast([P, npk, width])
                        nc.vector.tensor_tensor(
                            out=St[:, :].rearrange("p (a b) -> p a b", a=npk),
                            in0=iota_rep, in1=pcb, op=OP.is_equal)
                        stiles[k] = (St, pi0, width)
                    # gathers (phase-A order), then parity mask per window run
                    G = gpool.tile([P, nch * 2 * D], f16, tag="G")
                    off = 0
                    a = pl.a0
                    for (swb, wrows, n_s, fresh) in pl.wins:
                        j0 = off // P
                        k = n_s // P
                        nc.gpsimd.dma_gather(
                            out_ap=G[:, j0 * 2 * D:(j0 + k) * 2 * D].rearrange(
                                "p (j f) -> p j f", f=2 * D),
                            in_ap=T[swb:swb + wrows, :],
                            idxs_ap=idxAt[:, a:a + n_s // 16],
                            num_idxs=n_s, num_idxs_reg=n_s, elem_size=2 * D,
                            single_packet=False, queue_num=_next_q())
                        # parity mask for this run's chunks (in place)
                        pq = par2t[:, pl.pq0 + 2 * j0:pl.pq0 + 2 * (j0 + k)]\
                            .to_broadcast([P, 2 * k, D])
                        gv = G[:, j0 * 2 * D:(j0 + k) * 2 * D].rearrange(
                            "p (a b) -> p a b", a=2 * k)
                        nc.vector.tensor_tensor(
                            out=gv, in0=gv, in1=pq, op=OP.mult)
                        off += n_s
                        a += n_s // 16
                    # selection matmuls per dest window, then lhs copy + msg
                    for k in range(nwin):
                        width = min(WW, Lp - k * WW)
                        if k not in stiles:
                            nc.vector.memset(
                                lhsG[:, k * WW:k * WW + width], 0.0)
                        else:
                            (St, pi0, width_) = stiles[k]
                            pieces_k = [(pi, j) for pi, (j, kk)
                                        in enumerate(pl.pieces) if kk == k]
                            ps = psA.tile([2 * D, width], f32, tag="psA")
                            for i, (pi, j) in enumerate(pieces_k):
                                nc.tensor.matmul(
                                    out=ps[:, :],
                                    lhsT=G[:, j * 2 * D:(j + 1) * 2 * D],
                                    rhs=St[:, (pi - pi0) * width:
                                           (pi - pi0 + 1) * width],
                                    start=(i == 0),
                                    stop=(i == len(pieces_k) - 1))
                            nc.scalar.activation(
                                out=lhsG[:, k * WW:k * WW + width],
                                in_=ps[:, :], func=ACT.Copy)
                        # msg matmuls for chunks in this window
                        for w2 in pl.wchunks[k]:
                            ps2 = psB.tile([P, D], f32, tag="psB")
                            nc.tensor.matmul(
                                out=ps2[:, :],
                                lhsT=lhsG[:, w2 * P:(w2 + 1) * P],
                                rhs=wtg[:, wcol:wcol + D],
                                start=True, stop=False)
                            nc.tensor.matmul(
                                out=ps2[:, :],
                                lhsT=cntt[:, w2 * P:(w2 + 1) * P],
                                rhs=wtc[:, wcol:wcol + D],
                                start=False, stop=True)
                            msgt = msgp.tile([P, D], f16, tag="msg")
                            nc.scalar.activation(out=msgt[:, :], in_=ps2[:, :],
                                                 func=ACT.Copy)
                            nhalf = nw2 // 2
                            if w2 < nhalf:
                                o_ap = agin[pl.ag0 + w2 * P:
                                            pl.ag0 + (w2 + 1) * P, 0:D]
                            else:
                                w2b = w2 - nhalf
                                o_ap = agin[pl.ag0 + w2b * P:
                                            pl.ag0 + (w2b + 1) * P, D:2 * D]
                            nc.sync.dma_start(out=o_ap, in_=msgt[:, :])
                else:
                    nc.vector.memset(lhsG[:, :], 0.0)
                    for w2 in range(nw2):
                        ps2 = psB.tile([P, D], f32, tag="psB")
                        nc.tensor.matmul(
                            out=ps2[:, :],
                            lhsT=lhsG[:, w2 * P:(w2 + 1) * P],
                            rhs=wtg[:, wcol:wcol + D],
                            start=True, stop=False)
                        nc.tensor.matmul(
                            out=ps2[:, :],
                            lhsT=cntt[:, w2 * P:(w2 + 1) * P],
                            rhs=wtc[:, wcol:wcol + D],
                            start=False, stop=True)
                        msgt = msgp.tile([P, D], f16, tag="msg")
                        nc.scalar.activation(out=msgt[:, :], in_=ps2[:, :],
                                             func=ACT.Copy)
                        nhalf = nw2 // 2
                        if w2 < nhalf:
                            o_ap = agin[pl.ag0 + w2 * P:
                                        pl.ag0 + (w2 + 1) * P, 0:D]
                        else:
                            w2b = w2 - nhalf
                            o_ap = agin[pl.ag0 + w2b * P:
                                        pl.ag0 + (w2b + 1) * P, D:2 * D]
                        nc.sync.dma_start(out=o_ap, in_=msgt[:, :])
                nc.gpsimd.collective_compute(
                    "AllGather", OP.bypass, replica_groups=rg,
                    ins=[agin[pl.ag0:pl.ag0 + pl.pr, :]],
                    outs=[T[pl.tbp:pl.tbp + C * pl.pr, :]])
                emit_ro_windows(t)

            # ---------------- readout ----------------
            if skip_ro:
                zo = smp.tile([B, 2 * D], f32, tag="outt")
                nc.vector.memset(zo[:, :], 0.0)
                nc.sync.dma_start(out=out_x[:, :], in_=zo[:, :])
                if dump_T:
                    nc.sync.dma_start(out=tdump_x[:, :], in_=T[:, :])
            else:
                ro_stage = int(os.environ.get("KERNEL_RO_STAGE", "9"))

                def _zero_out():
                    zo = smp.tile([B, 2 * D], f32, tag="outt")
                    nc.vector.memset(zo[:, :], 0.0)
                    nc.sync.dma_start(out=out_x[:, :], in_=zo[:, :])
                    if dump_T:
                        nc.sync.dma_start(out=tdump_x[:, :], in_=T[:, :])

                # staging write + perm gather
                nc.sync.dma_start(
                    out=stg_t[0:ro["A_pad"], :].rearrange(
                        "(j p) f -> p j f", p=P),
                    in_=roGA[:, :].rearrange("p (j f) -> p j f", f=2 * D))
                R = constp.tile([P, NRCH * 2 * D], f16)
                nc.gpsimd.dma_gather(
                    out_ap=R[:, :].rearrange("p (j f) -> p j f", f=2 * D),
                    in_ap=stg_t[0:ro["A_pad"], :],
                    idxs_ap=idxCt[:, 0:(NRCH * P) // 16],
                    num_idxs=NRCH * P, num_idxs_reg=NRCH * P,
                    elem_size=2 * D, single_packet=False,
                    queue_num=_next_q())
                if ro_stage <= 1:
                    _zero_out()
                else:
                    # mask wrong parity halves in place: R *= par2 (sum form)
                    rv = R[:, :].rearrange("p (a b) -> p a b", a=2 * NRCH)
                    p2b = p2rot[:, :].to_broadcast([P, 2 * NRCH, D])
                    q2b = q2rot[:, :].to_broadcast([P, 2 * NRCH, D])
                    nc.vector.tensor_tensor(out=rv, in0=rv, in1=p2b,
                                            op=OP.mult)
                    # sum pool: S16 one-hot matmuls
                    S16 = constp.tile([P, NRCH * 16], f16)
                    i16t = io16[:, 0:16]
                    i16_rep = bass.AP(
                        i16t.tensor, i16t.offset,
                        [i16t.ap[0], [0, NRCH], [1, 16]])
                    rgb = rgidt[:, :].to_broadcast([P, NRCH, 16])
                    nc.vector.tensor_tensor(
                        out=S16[:, :].rearrange("p (a b) -> p a b", a=NRCH),
                        in0=i16_rep, in1=rgb, op=OP.is_equal)
                    ps_sum = psR.tile([B, 2 * D], f32)
                    for j in range(NRCH):
                        nc.tensor.matmul(
                            out=ps_sum[:, :],
                            lhsT=S16[:, j * 16:(j + 1) * 16],
                            rhs=R[:, j * 2 * D:(j + 1) * 2 * D],
                            start=(j == 0), stop=(j == NRCH - 1))
                    sumsw = smp.tile([B, 2 * D], f32, tag="sumsw")
                    nc.scalar.activation(out=sumsw[:, :], in_=ps_sum[:, :],
                                         func=ACT.Copy)
                    sums = smp.tile([B, D], f32, tag="sums")
                    nc.vector.tensor_tensor(
                        out=sums[:, :], in0=sumsw[:, 0:D],
                        in1=sumsw[:, D:2 * D], op=OP.add)
                if ro_stage == 2:
                    _zero_out()
                elif ro_stage > 2:
                    # max form in place: R -= q2 (pad/wrong halves -> -BIG)
                    nc.vector.tensor_tensor(out=rv, in0=rv, in1=q2b,
                                            op=OP.subtract)
                    # max pool: per-graph DVE max-tree + transpose + reduce
                    maxT = constp.tile([P, B], f16)
                    for g, (c0, kg) in enumerate(meta["graph_chunks"]):
                        if kg == 1:
                            mx_ap = R[:, c0 * 2 * D:(c0 + 1) * 2 * D]
                        else:
                            mx = smp.tile([P, 2 * D], f16, tag="mx")
                            nc.vector.tensor_tensor(
                                out=mx[:, :],
                                in0=R[:, c0 * 2 * D:(c0 + 1) * 2 * D],
                                in1=R[:, (c0 + 1) * 2 * D:(c0 + 2) * 2 * D],
                                op=OP.max)
                            for q_ in range(2, kg):
                                nc.vector.tensor_tensor(
                                    out=mx[:, :], in0=mx[:, :],
                                    in1=R[:, (c0 + q_) * 2 * D:
                                          (c0 + q_ + 1) * 2 * D],
                                    op=OP.max)
                            mx_ap = mx[:, :]
                        pst = psC.tile([2 * D, P], f16, tag="pst")
                        nc.tensor.transpose(
                            out=pst[:, :], in_=mx_ap, identity=ident[:, :])
                        nc.vector.reduce_max(
                            out=maxT[0:2 * D, g:g + 1], in_=pst[:, :],
                            axis=AX.X)
                    psmx = psC.tile([B, P], f16, tag="pst")
                    nc.tensor.transpose(
                        out=psmx[:, :], in_=maxT[:, :], identity=ident[:, :])
                    mxf = smp.tile([B, P], f16, tag="mxf")
                    nc.scalar.activation(out=mxf[:, :], in_=psmx[:, :],
                                         func=ACT.Copy)
                    pr = smp.tile([B, 2 * D], f32, tag="pr")
                    nc.vector.tensor_tensor(
                        out=pr[:, 0:D], in0=mxf[:, 0:D], in1=mxf[:, D:2 * D],
                        op=OP.max)
                    nc.vector.tensor_copy(out=pr[:, D:2 * D], in_=sums[:, :])
                    if ro_stage == 3:
                        nc.sync.dma_start(out=out_x[:, :], in_=pr[:, :])
                        if dump_T:
                            nc.sync.dma_start(out=tdump_x[:, :], in_=T[:, :])
                    else:
                        nc.sync.dma_start(
                            out=prin[:, :].rearrange("(h g) f -> g h f", g=B),
                            in_=pr[:, :].rearrange("g (h f) -> g h f", h=2))
                        nc.gpsimd.collective_compute(
                            "AllGather", OP.bypass, replica_groups=rg,
                            ins=[prin[:, :]], outs=[prout[:, :]])
                        pr3 = prout[:, :].rearrange("(r gg) f -> gg r f", r=C)
                        mx8 = smp.tile([B, C * D], f32, tag="mx8")
                        sm8 = smp.tile([B, C * D], f32, tag="sm8")
                        nc.sync.dma_start(
                            out=mx8[:, :].rearrange("g (r f) -> g r f", f=D),
                            in_=pr3[0:B])
                        nc.sync.dma_start(
                            out=sm8[:, :].rearrange("g (r f) -> g r f", f=D),
                            in_=pr3[B:2 * B])
                        outt = smp.tile([B, 2 * D], f32, tag="outt")
                        t1m = smp.tile([B, 4 * D], f32, tag="t1m")
                        t2m = smp.tile([B, 2 * D], f32, tag="t2m")
                        nc.vector.tensor_tensor(
                            out=t1m[:, :], in0=mx8[:, 0:4 * D],
                            in1=mx8[:, 4 * D:8 * D], op=OP.max)
                        nc.vector.tensor_tensor(
                            out=t2m[:, :], in0=t1m[:, 0:2 * D],
                            in1=t1m[:, 2 * D:4 * D], op=OP.max)
                        nc.vector.tensor_tensor(
                            out=outt[:, 0:D], in0=t2m[:, 0:D],
                            in1=t2m[:, D:2 * D], op=OP.max)
                        t1s = smp.tile([B, 4 * D], f32, tag="t1s")
                        t2s = smp.tile([B, 2 * D], f32, tag="t2s")
                        nc.vector.tensor_tensor(
                            out=t1s[:, :], in0=sm8[:, 0:4 * D],
                            in1=sm8[:, 4 * D:8 * D], op=OP.add)
                        nc.vector.tensor_tensor(
                            out=t2s[:, :], in0=t1s[:, 0:2 * D],
                            in1=t1s[:, 2 * D:4 * D], op=OP.add)
                        nc.vector.tensor_tensor(
                            out=outt[:, D:2 * D], in0=t2s[:, 0:D],
                            in1=t2s[:, D:2 * D], op=OP.add)
                        nc.sync.dma_start(out=out_x[:, :], in_=outt[:, :])
                        if dump_T:
                            nc.sync.dma_start(out=tdump_x[:, :], in_=T[:, :])
    nc.compile()
    return nc


def _in_maps(meta, arrays):
    maps = []
    for c in range(C):
        maps.append(dict(
            tab=arrays["tab"],
            wtG=arrays["wtG"],
            wtC=arrays["wtC"],
            iota=arrays["iota512"],
            iota16=arrays["iota16"],
            idxA=np.ascontiguousarray(arrays["idxA"][c]),
            idxC=np.ascontiguousarray(arrays["idxC"][c]),
            pcd=np.ascontiguousarray(arrays["pcd"][c]),
            par2=np.ascontiguousarray(arrays["par2"][c]),
            cnts=np.ascontiguousarray(arrays["cnts"][c]),
            rgid=np.ascontiguousarray(arrays["rgid"][c]),
            par2ro=np.ascontiguousarray(arrays["par2ro"][c]),
            q2ro=np.ascontiguousarray(arrays["q2ro"][c]),
        ))
    return maps


_LAST_RESULTS = None  # stash for test harness (exec time, trace)


def kernel(**inputs):
    global _LAST_RESULTS
    import os
    meta, arrays = _preprocess(**inputs)
    nc = _build(meta)
    from concourse.bass_utils import run_bass_kernel_spmd
    res = run_bass_kernel_spmd(nc, _in_maps(meta, arrays),
                               core_ids=list(range(C)),
                               trace=bool(os.environ.get("KERNEL_TRACE")))
    _LAST_RESULTS = res
    return np.asarray(res.results[0]["out"])
